# revision 42
# baseline (speedup 1.0000x reference)
"""MoE model (embed -> gate -> 4 dense experts -> softmax combine) on 8 TRN2 cores.

Data-parallel: batch (65536 tokens) sharded 8192/core; embedding tables,
expert weights, and gating weights replicated on every core (SBUF-resident,
bf16). All on-chip activations are kept feature-major ("transposed") so that
every matmul consumes operands in their natural layout:

  e_T[f, t]   = one-hot(vocab) matmul against the embedding tables
  h_T[d, t]   = silu(W1[e].T-tiles @ e_T + b1)        (PSUM fp32, evac bf16)
  eo_T[o, t]  = W2[e].T-tiles @ h_T + b2              (PSUM fp32)
  logits[e,t] = Wg.T-tiles @ e_T + bg ; softmax via exp / sum (unnormalized
                weights combined first, one reciprocal row scale at the end)
  out_T[o, t] = (sum_e exp_e * eo_e) * recip          (DVE, fp32)

Output per core is [128, 8192] (feature-major); host transposes on unshard.
"""

import os
import numpy as np
import ml_dtypes

import concourse.bass as bass
import concourse.mybir as mybir
import concourse.tile as tile
from concourse.vector_clock import ScopedClock, VectorClock
from concourse.bass_utils import run_bass_kernel_spmd

BF16 = ml_dtypes.bfloat16

B = 65536
V = 512
D = 1024
IN = 2048
E = 4
OUT = 128
NCORES = 8
BL = B // NCORES          # tokens per core
ST = 512                  # tokens per supertile (max PSUM free dim, fp32)
NST = BL // ST            # supertiles per core
KC = IN // 128            # 16 feature chunks
DC = D // 128             # 8 hidden chunks
VC = V // 128             # 4 vocab chunks

LAST_EXEC_NS = None       # set when BASSMOE_TRACE=1


class _TC(tile.TileContext):
    """Unmodified TileContext; kept as a named subclass for clarity."""


def _legalize_waits(nc, max_waits=1):
    """This walrus build rejects instructions carrying more than ~1 sync-wait
    command ("Too many sync wait commands", CoreV2/V3GenImpl setupSyncWait).
    Hoist all but the last wait of every instruction onto single-wait NoOps
    placed immediately before it in the same engine's stream."""
    for f in nc.m.functions:
        for bb in f.blocks:
            insts = bb.instructions
            if not any(
                inst.sync_info is not None and len(inst.sync_info.on_wait) > max_waits
                for inst in insts
            ):
                continue
            new = []
            for inst in insts:
                si = inst.sync_info
                waits = list(si.on_wait) if si is not None else []
                if len(waits) > max_waits:
                    for w in waits[:-max_waits]:
                        nop = mybir.InstNoOp(
                            name=f"legw-{nc.next_id()}", ins=[], outs=[]
                        )
                        nop.engine = inst.engine
                        nop.sync_info = mybir.SyncInfo(on_wait=[w], on_update=[])
                        new.append(nop)
                    inst.sync_info = mybir.SyncInfo(
                        on_wait=waits[-max_waits:], on_update=list(si.on_update)
                    )
                new.append(inst)
            bb.instructions = new


def build_program(nst=NST, legalize=True, gather=True):
    dt = mybir.dt
    f32, bf16, f16 = dt.float32, dt.bfloat16, dt.float16
    AF = mybir.ActivationFunctionType
    ALU = mybir.AluOpType

    nc = bass.Bass()

    if gather:
        # token index tiles in dma_gather's wrapped layout: idx j at [j%16, j//16],
        # replicated across the 8 gpsimd cores (8 x 16 = 128 partitions)
        x0d = nc.dram_tensor("x0i", [nst, 128, ST // 16], dt.int16, kind="ExternalInput")
        x1d = nc.dram_tensor("x1i", [nst, 128, ST // 16], dt.int16, kind="ExternalInput")
        embd = nc.dram_tensor("embg", [2, V, D], bf16, kind="ExternalInput")
    else:
        x0d = nc.dram_tensor("x0", [nst, 1, ST], f16, kind="ExternalInput")
        x1d = nc.dram_tensor("x1", [nst, 1, ST], f16, kind="ExternalInput")
        embd = nc.dram_tensor("embs", [128, 2, VC, DC, 128], bf16, kind="ExternalInput")
    w1d = nc.dram_tensor("w1s", [128, E, KC, DC, 128], bf16, kind="ExternalInput")
    w2d = nc.dram_tensor("w2s", [128, E, DC, OUT], bf16, kind="ExternalInput")
    wgd = nc.dram_tensor("wgs", [128, KC, E], bf16, kind="ExternalInput")
    b1d = nc.dram_tensor("b1s", [128, E, DC], f32, kind="ExternalInput")
    b2d = nc.dram_tensor("b2s", [128, E], f32, kind="ExternalInput")
    bgd = nc.dram_tensor("bgs", [E, 1], f32, kind="ExternalInput")
    if not gather:
        ivd = nc.dram_tensor("ivs", [128, VC], f32, kind="ExternalInput")
    seld = nc.dram_tensor("sels", [E, E, 128], bf16, kind="ExternalInput")
    outd = nc.dram_tensor("out", [128, nst * ST], f32, kind="ExternalOutput")

    with _TC(nc) as tc:
        with (
            tc.tile_pool(name="const", bufs=1) as cpool,
            tc.tile_pool(name="xt", bufs=2) as xpool,
            tc.tile_pool(name="mask", bufs=1) as mpool,
            tc.tile_pool(name="et", bufs=2 if gather else 1) as etpool,
            tc.tile_pool(name="hs", bufs=1) as hpool,
            tc.tile_pool(name="sm", bufs=2) as smpool,
            tc.tile_pool(name="gsc", bufs=1) as gspool,
            tc.tile_pool(name="sgp", bufs=2) as sgpool,
            tc.tile_pool(name="accp", bufs=1) as apool,
            tc.tile_pool(name="outp", bufs=2) as opool,
            tc.tile_pool(name="pmm", bufs=2, space="PSUM") as pmm,
            tc.tile_pool(name="peo", bufs=2, space="PSUM") as peo,
            tc.tile_pool(name="prb", bufs=2, space="PSUM") as prb,
            tc.tile_pool(name="pmisc", bufs=2, space="PSUM") as pmisc,
        ):
            # --- resident weights / constants ---
            if not gather:
                emb_sb = cpool.tile([128, 2, VC, DC, 128], bf16)
                nc.sync.dma_start(emb_sb[:], embd[:])
            w1_sb = cpool.tile([128, E, KC, DC, 128], bf16)
            nc.sync.dma_start(w1_sb[:], w1d[:])
            w2_sb = cpool.tile([128, E, DC, OUT], bf16)
            nc.sync.dma_start(w2_sb[:], w2d[:])
            wg_sb = cpool.tile([128, KC, E], bf16)
            nc.sync.dma_start(wg_sb[:], wgd[:])
            b1_sb = cpool.tile([128, E, DC], f32)
            nc.sync.dma_start(b1_sb[:], b1d[:])
            b2_sb = cpool.tile([128, E], f32)
            nc.sync.dma_start(b2_sb[:], b2d[:])
            bg_sb = cpool.tile([E, 1], f32)
            nc.sync.dma_start(bg_sb[:], bgd[:])
            if not gather:
                iv_sb = cpool.tile([128, VC], f32)
                nc.sync.dma_start(iv_sb[:], ivd[:])
                ones_f16 = cpool.tile([1, 128], f16)
                nc.vector.memset(ones_f16[:], 1.0)
            sel_sb = cpool.tile([E, E, 128], bf16)
            nc.sync.dma_start(sel_sb[:], seld[:])

            ones4_bf = cpool.tile([E, 1], bf16)
            nc.vector.memset(ones4_bf[:], 1.0)
            ones128_bf = cpool.tile([1, 128], bf16)
            nc.vector.memset(ones128_bf[:], 1.0)

            if gather:
                from concourse import library_config

                nc.gpsimd.load_library(library_config.mlp)

            for i in range(nst):
                eT = etpool.tile([128, KC, ST], bf16, tag="eT")
                if gather:
                    # --- embedding lookup via transposing gather DMA ---
                    for tbl, xd in enumerate((x0d, x1d)):
                        xi = xpool.tile([128, ST // 16], dt.int16, tag=f"xi{tbl}")
                        nc.sync.dma_start(xi[:], xd[i])
                        nc.gpsimd.dma_gather(
                            out_ap=eT[:, tbl * DC : (tbl + 1) * DC, :],
                            in_ap=embd[tbl],
                            idxs_ap=xi[:],
                            num_idxs=ST,
                            num_idxs_reg=ST,
                            elem_size=D,
                            transpose=True,
                        )
                else:
                    # --- x broadcast across partitions (K=1 matmul) ---
                    x0_sb = xpool.tile([1, ST], f16, tag="x0")
                    nc.sync.dma_start(x0_sb[:], x0d[i])
                    x1_sb = xpool.tile([1, ST], f16, tag="x1")
                    nc.sync.dma_start(x1_sb[:], x1d[i])
                    xb = []
                    for tbl, xs in enumerate((x0_sb, x1_sb)):
                        p = pmisc.tile([128, ST], f32, tag="misc")
                        nc.tensor.matmul(p[:], ones_f16[:], xs[:])
                        xb.append(p)

                    # --- one-hot masks + embedding matmul -> e_T ---
                    for tbl in range(2):
                        masks = []
                        for vc in range(VC):
                            m = mpool.tile([128, ST], bf16, tag=f"m{vc}")
                            nc.vector.tensor_scalar(
                                m[:], xb[tbl][:], iv_sb[:, vc : vc + 1], None,
                                ALU.is_equal,
                            )
                            masks.append(m)
                        for dc in range(DC):
                            ps = pmm.tile([128, ST], f32, tag="mm")
                            for vc in range(VC):
                                nc.tensor.matmul(
                                    ps[:],
                                    emb_sb[:, tbl, vc, dc, :],
                                    masks[vc][:],
                                    start=(vc == 0),
                                    stop=(vc == VC - 1),
                                )
                            nc.scalar.copy(eT[:, tbl * DC + dc, :], ps[:])

                # --- gating: logits -> exp -> sum -> reciprocal bcast ---
                lp = pmisc.tile([E, ST], f32, tag="misc")
                for kc in range(KC):
                    nc.tensor.matmul(
                        lp[:],
                        wg_sb[:, kc, :],
                        eT[:, kc, :],
                        start=(kc == 0),
                        stop=(kc == KC - 1),
                    )
                expt = smpool.tile([E, ST], bf16, tag="expt")
                nc.scalar.activation(expt[:], lp[:], AF.Exp, bias=bg_sb[:])
                sp = pmisc.tile([1, ST], f32, tag="misc")
                nc.tensor.matmul(sp[:], ones4_bf[:], expt[:])
                rec = smpool.tile([1, ST], f32, tag="rec")
                nc.vector.reciprocal(rec[:], sp[:])
                recb = smpool.tile([1, ST], bf16, tag="recb")
                nc.vector.tensor_copy(recb[:], rec[:])
                rbp = prb.tile([128, ST], f32, tag="rb")
                nc.tensor.matmul(rbp[:], ones128_bf[:], recb[:])

                # --- experts ---
                acc = apool.tile([128, ST], f32, tag="acc")
                for e in range(E):
                    hs = hpool.tile([128, DC, ST], bf16, tag="hs")
                    for dc in range(DC):
                        hp = pmm.tile([128, ST], f32, tag="mm")
                        for kc in range(KC):
                            nc.tensor.matmul(
                                hp[:],
                                w1_sb[:, e, kc, dc, :],
                                eT[:, kc, :],
                                start=(kc == 0),
                                stop=(kc == KC - 1),
                            )
                        sg = sgpool.tile([128, ST], f32, tag="sg")
                        nc.scalar.activation(
                            sg[:], hp[:], AF.Sigmoid, bias=b1_sb[:, e, dc : dc + 1]
                        )
                        nc.vector.scalar_tensor_tensor(
                            hs[:, dc, :], hp[:], b1_sb[:, e, dc : dc + 1], sg[:],
                            ALU.add, ALU.mult,
                        )
                    eop = peo.tile([128, ST], f32, tag="eo")
                    for dc in range(DC):
                        nc.tensor.matmul(
                            eop[:],
                            w2_sb[:, e, dc, :],
                            hs[:, dc, :],
                            start=(dc == 0),
                            stop=(dc == DC - 1),
                        )
                    gp = pmisc.tile([128, ST], f32, tag="misc")
                    nc.tensor.matmul(gp[:], sel_sb[:, e, :], expt[:])
                    gs = gspool.tile([128, ST], f32, tag="gs")
                    nc.scalar.copy(gs[:], gp[:])
                    if e == 0:
                        nc.vector.scalar_tensor_tensor(
                            acc[:], eop[:], b2_sb[:, e : e + 1], gs[:], ALU.add, ALU.mult
                        )
                    else:
                        tmp = opool.tile([128, ST], f32, tag="outt")
                        nc.vector.scalar_tensor_tensor(
                            tmp[:], eop[:], b2_sb[:, e : e + 1], gs[:], ALU.add, ALU.mult
                        )
                        nc.vector.tensor_add(acc[:], acc[:], tmp[:])

                outt = opool.tile([128, ST], f32, tag="outt")
                nc.vector.tensor_tensor(outt[:], acc[:], rbp[:], ALU.mult)
                nc.sync.dma_start(outd[:, i * ST : (i + 1) * ST], outt[:])

    if legalize:
        _legalize_waits(nc)
    # populate .instr bytes for extended-ISA instructions (library reload for
    # dma_gather) — raw Bass skips Bacc's codegen pass; walrus errors with
    # "ISA wrong length" on empty instr otherwise
    mybir.codegen_inst_isa_subclasses(nc)
    return nc


def marshal_inputs(
    x, emb0, emb1, W1, b1, W2, b2, Wg, bg, nst=NST, ncores=NCORES, gather=True
):
    """Host-side: cast/reshape full inputs into per-core in_maps."""
    n_tok = ncores * nst * ST
    if gather:
        # wrapped int16 index layout: token j of a supertile at [j%16, j//16],
        # tiled 8x across partitions for the 8 gpsimd cores
        def _wrap(col):
            w = (
                col[:n_tok].astype(np.int16).reshape(ncores, nst, ST // 16, 16)
                .transpose(0, 1, 3, 2)
            )
            return np.ascontiguousarray(np.tile(w, (1, 1, 8, 1)))

        x0h = _wrap(x[:, 0])
        x1h = _wrap(x[:, 1])
        embs = np.ascontiguousarray(np.stack([emb0, emb1]).astype(BF16))
        xkeys = ("x0i", "x1i")
        embkey = "embg"
    else:
        x0h = np.ascontiguousarray(
            x[:n_tok, 0].astype(np.float16).reshape(ncores, nst, 1, ST)
        )
        x1h = np.ascontiguousarray(
            x[:n_tok, 1].astype(np.float16).reshape(ncores, nst, 1, ST)
        )
        embs = np.ascontiguousarray(
            np.stack([emb0, emb1])
            .reshape(2, VC, 128, DC, 128)
            .transpose(2, 0, 1, 3, 4)
            .astype(BF16)
        )
        xkeys = ("x0", "x1")
        embkey = "embs"
    w1s = np.ascontiguousarray(
        np.asarray(W1).reshape(E, KC, 128, DC, 128).transpose(2, 0, 1, 3, 4).astype(BF16)
    )
    w2s = np.ascontiguousarray(
        np.asarray(W2).reshape(E, DC, 128, OUT).transpose(2, 0, 1, 3).astype(BF16)
    )
    wgs = np.ascontiguousarray(
        np.asarray(Wg).reshape(KC, 128, E).transpose(1, 0, 2).astype(BF16)
    )
    b1s = np.ascontiguousarray(
        np.asarray(b1).reshape(E, DC, 128).transpose(2, 0, 1).astype(np.float32)
    )
    b2s = np.ascontiguousarray(np.asarray(b2).T.astype(np.float32))
    bgs = np.ascontiguousarray(np.asarray(bg).reshape(E, 1).astype(np.float32))
    ivs = np.ascontiguousarray(
        (np.arange(VC)[None, :] * 128 + np.arange(128)[:, None]).astype(np.float32)
    )
    sels = np.ascontiguousarray(
        np.broadcast_to(np.eye(E, dtype=np.float32)[:, :, None], (E, E, 128)).astype(
            BF16
        )
    )
    shared = {
        embkey: embs, "w1s": w1s, "w2s": w2s, "wgs": wgs,
        "b1s": b1s, "b2s": b2s, "bgs": bgs, "sels": sels,
    }
    if not gather:
        shared["ivs"] = ivs
    return [{xkeys[0]: x0h[c], xkeys[1]: x1h[c], **shared} for c in range(ncores)]


def kernel(x, emb0, emb1, W1, b1, W2, b2, Wg, bg):
    global LAST_EXEC_NS
    nc = build_program()
    in_maps = marshal_inputs(x, emb0, emb1, W1, b1, W2, b2, Wg, bg)
    trace = os.environ.get("BASSMOE_TRACE", "0") == "1"
    res = run_bass_kernel_spmd(nc, in_maps, list(range(NCORES)), trace=trace)
    LAST_EXEC_NS = res.exec_time_ns
    out = np.empty((B, OUT), dtype=np.float32)
    for c in range(NCORES):
        out[c * BL : (c + 1) * BL, :] = res.results[c]["out"].T
    return out


# revision 52
# speedup vs baseline: 1.0691x; 1.0691x over previous
"""MoE model (embed -> gate -> 4 dense experts -> softmax combine) on 8 TRN2 cores.

Data-parallel: batch (65536 tokens) sharded 8192/core; embedding tables,
expert weights, and gating weights replicated on every core (SBUF-resident,
bf16). All on-chip activations are kept feature-major ("transposed") so that
every matmul consumes operands in their natural layout:

  e_T[f, t]   = one-hot(vocab) matmul against the embedding tables
  h_T[d, t]   = silu(W1[e].T-tiles @ e_T + b1)        (PSUM fp32, evac bf16)
  eo_T[o, t]  = W2[e].T-tiles @ h_T + b2              (PSUM fp32)
  logits[e,t] = Wg.T-tiles @ e_T + bg ; softmax via exp / sum (unnormalized
                weights combined first, one reciprocal row scale at the end)
  out_T[o, t] = (sum_e exp_e * eo_e) * recip          (DVE, fp32)

Output per core is [128, 8192] (feature-major); host transposes on unshard.
"""

import os
import numpy as np
import ml_dtypes

import concourse.bass as bass
import concourse.mybir as mybir
import concourse.tile as tile
from concourse.vector_clock import ScopedClock, VectorClock
from concourse.bass_utils import run_bass_kernel_spmd

BF16 = ml_dtypes.bfloat16

B = 65536
V = 512
D = 1024
IN = 2048
E = 4
OUT = 128
NCORES = 8
BL = B // NCORES          # tokens per core
ST = 512                  # tokens per supertile (max PSUM free dim, fp32)
NST = BL // ST            # supertiles per core
KC = IN // 128            # 16 feature chunks
DC = D // 128             # 8 hidden chunks
VC = V // 128             # 4 vocab chunks

LAST_EXEC_NS = None       # set when BASSMOE_TRACE=1


class _TC(tile.TileContext):
    """Unmodified TileContext; kept as a named subclass for clarity."""


def _legalize_waits(nc, max_waits=1):
    """This walrus build rejects instructions carrying more than ~1 sync-wait
    command ("Too many sync wait commands", CoreV2/V3GenImpl setupSyncWait).
    Hoist all but the last wait of every instruction onto single-wait NoOps
    placed immediately before it in the same engine's stream."""
    for f in nc.m.functions:
        for bb in f.blocks:
            insts = bb.instructions
            if not any(
                inst.sync_info is not None and len(inst.sync_info.on_wait) > max_waits
                for inst in insts
            ):
                continue
            new = []
            for inst in insts:
                si = inst.sync_info
                waits = list(si.on_wait) if si is not None else []
                if len(waits) > max_waits:
                    for w in waits[:-max_waits]:
                        nop = mybir.InstNoOp(
                            name=f"legw-{nc.next_id()}", ins=[], outs=[]
                        )
                        nop.engine = inst.engine
                        nop.sync_info = mybir.SyncInfo(on_wait=[w], on_update=[])
                        new.append(nop)
                    inst.sync_info = mybir.SyncInfo(
                        on_wait=waits[-max_waits:], on_update=list(si.on_update)
                    )
                new.append(inst)
            bb.instructions = new


def build_program(nst=NST, legalize=True, gather=False):
    dt = mybir.dt
    f32, bf16, f16 = dt.float32, dt.bfloat16, dt.float16
    AF = mybir.ActivationFunctionType
    ALU = mybir.AluOpType

    nc = bass.Bass()

    if gather:
        # token index tiles in dma_gather's wrapped layout: idx j at [j%16, j//16],
        # replicated across the 8 gpsimd cores (8 x 16 = 128 partitions)
        x0d = nc.dram_tensor("x0i", [nst, 128, ST // 16], dt.int16, kind="ExternalInput")
        x1d = nc.dram_tensor("x1i", [nst, 128, ST // 16], dt.int16, kind="ExternalInput")
        embd = nc.dram_tensor("embg", [2, V, D], bf16, kind="ExternalInput")
    else:
        x0d = nc.dram_tensor("x0", [nst, 1, ST], f16, kind="ExternalInput")
        x1d = nc.dram_tensor("x1", [nst, 1, ST], f16, kind="ExternalInput")
        embd = nc.dram_tensor("embs", [128, 2, VC, DC, 128], bf16, kind="ExternalInput")
    w1d = nc.dram_tensor("w1s", [E, 128, KC, DC, 128], bf16, kind="ExternalInput")
    w2d = nc.dram_tensor("w2s", [128, E, DC, OUT], bf16, kind="ExternalInput")
    wgd = nc.dram_tensor("wgs", [128, KC, E], bf16, kind="ExternalInput")
    b1d = nc.dram_tensor("b1s", [128, E, DC], f32, kind="ExternalInput")
    b2d = nc.dram_tensor("b2s", [128, E], f32, kind="ExternalInput")
    bgd = nc.dram_tensor("bgs", [E, 1], f32, kind="ExternalInput")
    if not gather:
        ivd = nc.dram_tensor("ivs", [128, VC], f32, kind="ExternalInput")
    seld = nc.dram_tensor("sels", [E, E, 128], bf16, kind="ExternalInput")
    outd = nc.dram_tensor("out", [128, nst * ST], f32, kind="ExternalOutput")

    with _TC(nc) as tc:
        with (
            tc.tile_pool(name="const", bufs=1) as cpool,
            tc.tile_pool(name="xt", bufs=2) as xpool,
            tc.tile_pool(name="mask", bufs=1) as mpool,
            tc.tile_pool(name="et", bufs=2 if gather else 1) as etpool,
            tc.tile_pool(name="hs", bufs=1) as hpool,
            tc.tile_pool(name="sm", bufs=1) as smpool,
            tc.tile_pool(name="gsc", bufs=1) as gspool,
            tc.tile_pool(name="sgp", bufs=2) as sgpool,
            tc.tile_pool(name="accp", bufs=1) as apool,
            tc.tile_pool(name="outp", bufs=2) as opool,
            tc.tile_pool(name="pmm", bufs=2, space="PSUM") as pmm,
            tc.tile_pool(name="peo", bufs=2, space="PSUM") as peo,
            tc.tile_pool(name="prb", bufs=2, space="PSUM") as prb,
            tc.tile_pool(name="pmisc", bufs=2, space="PSUM") as pmisc,
        ):
            # --- resident weights / constants (order = DMA priority: the
            # first supertile needs emb/wg immediately, w1[e] at ~20us) ---
            if not gather:
                emb_sb = cpool.tile([128, 2, VC, DC, 128], bf16)
                nc.sync.dma_start(emb_sb[:], embd[:])
                iv_sb = cpool.tile([128, VC], f32)
                nc.sync.dma_start(iv_sb[:], ivd[:])
                ones_f16 = cpool.tile([1, 128], f16)
                nc.vector.memset(ones_f16[:], 1.0)
            wg_sb = cpool.tile([128, KC, E], bf16)
            nc.sync.dma_start(wg_sb[:], wgd[:])
            b1_sb = cpool.tile([128, E, DC], f32)
            nc.sync.dma_start(b1_sb[:], b1d[:])
            b2_sb = cpool.tile([128, E], f32)
            nc.sync.dma_start(b2_sb[:], b2d[:])
            bg_sb = cpool.tile([E, 1], f32)
            nc.sync.dma_start(bg_sb[:], bgd[:])
            sel_sb = cpool.tile([E, E, 128], bf16)
            nc.sync.dma_start(sel_sb[:], seld[:])
            w2_sb = cpool.tile([128, E, DC, OUT], bf16)
            nc.sync.dma_start(w2_sb[:], w2d[:])
            w1_sbs = []
            for e in range(E):
                t = cpool.tile([128, KC, DC, 128], bf16, tag=f"w1e{e}")
                nc.sync.dma_start(t[:], w1d[e])
                w1_sbs.append(t)

            ones4_bf = cpool.tile([E, 1], bf16)
            nc.vector.memset(ones4_bf[:], 1.0)
            ones128_bf = cpool.tile([1, 128], bf16)
            nc.vector.memset(ones128_bf[:], 1.0)

            if gather:
                from concourse import library_config

                nc.gpsimd.load_library(library_config.mlp)

            def build_masks(i):
                """x-broadcast (K=1 matmul) + one-hot compares for supertile i."""
                x0_sb = xpool.tile([1, ST], f16, tag="x0")
                nc.sync.dma_start(x0_sb[:], x0d[i])
                x1_sb = xpool.tile([1, ST], f16, tag="x1")
                nc.sync.dma_start(x1_sb[:], x1d[i])
                ms = []
                for tbl, xs in enumerate((x0_sb, x1_sb)):
                    p = pmisc.tile([128, ST], f32, tag="misc")
                    nc.tensor.matmul(p[:], ones_f16[:], xs[:])
                    row = []
                    for vc in range(VC):
                        m = mpool.tile([128, ST], bf16, tag=f"m{tbl}{vc}")
                        nc.vector.tensor_scalar(
                            m[:], p[:], iv_sb[:, vc : vc + 1], None, ALU.is_equal
                        )
                        row.append(m)
                    ms.append(row)
                return ms

            cur_masks = None if gather else build_masks(0)

            for i in range(nst):
                eT = etpool.tile([128, KC, ST], bf16, tag="eT")
                if gather:
                    # --- embedding lookup via transposing gather DMA ---
                    for tbl, xd in enumerate((x0d, x1d)):
                        xi = xpool.tile([128, ST // 16], dt.int16, tag=f"xi{tbl}")
                        nc.sync.dma_start(xi[:], xd[i])
                        nc.gpsimd.dma_gather(
                            out_ap=eT[:, tbl * DC : (tbl + 1) * DC, :],
                            in_ap=embd[tbl],
                            idxs_ap=xi[:],
                            num_idxs=ST,
                            num_idxs_reg=ST,
                            elem_size=D,
                            transpose=True,
                        )
                else:
                    # --- one-hot embedding matmul -> e_T ---
                    for tbl in range(2):
                        for dc in range(DC):
                            ps = pmm.tile([128, ST], f32, tag="mm")
                            for vc in range(VC):
                                nc.tensor.matmul(
                                    ps[:],
                                    emb_sb[:, tbl, vc, dc, :],
                                    cur_masks[tbl][vc][:],
                                    start=(vc == 0),
                                    stop=(vc == VC - 1),
                                )
                            nc.scalar.copy(eT[:, tbl * DC + dc, :], ps[:])

                # --- gating: logits -> exp -> sum -> reciprocal bcast ---
                lp = pmisc.tile([E, ST], f32, tag="misc")
                for kc in range(KC):
                    nc.tensor.matmul(
                        lp[:],
                        wg_sb[:, kc, :],
                        eT[:, kc, :],
                        start=(kc == 0),
                        stop=(kc == KC - 1),
                    )
                expt = smpool.tile([E, ST], bf16, tag="expt")
                nc.scalar.activation(expt[:], lp[:], AF.Exp, bias=bg_sb[:])
                sp = pmisc.tile([1, ST], f32, tag="misc")
                nc.tensor.matmul(sp[:], ones4_bf[:], expt[:])
                rec = smpool.tile([1, ST], f32, tag="rec")
                nc.vector.reciprocal(rec[:], sp[:])
                recb = smpool.tile([1, ST], bf16, tag="recb")
                nc.vector.tensor_copy(recb[:], rec[:])
                rbp = prb.tile([128, ST], f32, tag="rb")
                nc.tensor.matmul(rbp[:], ones128_bf[:], recb[:])

                # masks for the next supertile: the 2 tiny PE matmuls run now,
                # the DVE compares overlap with the expert phase below, so the
                # next e-build starts without a cross-engine stall
                if not gather and i + 1 < nst:
                    next_masks = build_masks(i + 1)

                # --- experts ---
                acc = apool.tile([128, ST], f32, tag="acc")
                for e in range(E):
                    hs = hpool.tile([128, DC, ST], bf16, tag="hs")
                    for dc in range(DC):
                        hp = pmm.tile([128, ST], f32, tag="mm")
                        for kc in range(KC):
                            nc.tensor.matmul(
                                hp[:],
                                w1_sbs[e][:, kc, dc, :],
                                eT[:, kc, :],
                                start=(kc == 0),
                                stop=(kc == KC - 1),
                            )
                        sg = sgpool.tile([128, ST], f32, tag="sg")
                        nc.scalar.activation(
                            sg[:], hp[:], AF.Sigmoid, bias=b1_sb[:, e, dc : dc + 1]
                        )
                        nc.vector.scalar_tensor_tensor(
                            hs[:, dc, :], hp[:], b1_sb[:, e, dc : dc + 1], sg[:],
                            ALU.add, ALU.mult,
                        )
                    eop = peo.tile([128, ST], f32, tag="eo")
                    for dc in range(DC):
                        nc.tensor.matmul(
                            eop[:],
                            w2_sb[:, e, dc, :],
                            hs[:, dc, :],
                            start=(dc == 0),
                            stop=(dc == DC - 1),
                        )
                    gp = pmisc.tile([128, ST], f32, tag="misc")
                    nc.tensor.matmul(gp[:], sel_sb[:, e, :], expt[:])
                    gs = gspool.tile([128, ST], f32, tag="gs")
                    nc.scalar.copy(gs[:], gp[:])
                    if e == 0:
                        nc.vector.scalar_tensor_tensor(
                            acc[:], eop[:], b2_sb[:, e : e + 1], gs[:], ALU.add, ALU.mult
                        )
                    else:
                        tmp = opool.tile([128, ST], f32, tag="outt")
                        nc.vector.scalar_tensor_tensor(
                            tmp[:], eop[:], b2_sb[:, e : e + 1], gs[:], ALU.add, ALU.mult
                        )
                        nc.vector.tensor_add(acc[:], acc[:], tmp[:])

                outt = opool.tile([128, ST], f32, tag="outt")
                nc.vector.tensor_tensor(outt[:], acc[:], rbp[:], ALU.mult)
                nc.sync.dma_start(outd[:, i * ST : (i + 1) * ST], outt[:])
                if not gather and i + 1 < nst:
                    cur_masks = next_masks

    if legalize:
        _legalize_waits(nc)
    # populate .instr bytes for extended-ISA instructions (library reload for
    # dma_gather) — raw Bass skips Bacc's codegen pass; walrus errors with
    # "ISA wrong length" on empty instr otherwise
    mybir.codegen_inst_isa_subclasses(nc)
    return nc


def marshal_inputs(
    x, emb0, emb1, W1, b1, W2, b2, Wg, bg, nst=NST, ncores=NCORES, gather=False
):
    """Host-side: cast/reshape full inputs into per-core in_maps."""
    n_tok = ncores * nst * ST
    if gather:
        # wrapped int16 index layout: token j of a supertile at [j%16, j//16],
        # tiled 8x across partitions for the 8 gpsimd cores
        def _wrap(col):
            w = (
                col[:n_tok].astype(np.int16).reshape(ncores, nst, ST // 16, 16)
                .transpose(0, 1, 3, 2)
            )
            return np.ascontiguousarray(np.tile(w, (1, 1, 8, 1)))

        x0h = _wrap(x[:, 0])
        x1h = _wrap(x[:, 1])
        embs = np.ascontiguousarray(np.stack([emb0, emb1]).astype(BF16))
        xkeys = ("x0i", "x1i")
        embkey = "embg"
    else:
        x0h = np.ascontiguousarray(
            x[:n_tok, 0].astype(np.float16).reshape(ncores, nst, 1, ST)
        )
        x1h = np.ascontiguousarray(
            x[:n_tok, 1].astype(np.float16).reshape(ncores, nst, 1, ST)
        )
        embs = np.ascontiguousarray(
            np.stack([emb0, emb1])
            .reshape(2, VC, 128, DC, 128)
            .transpose(2, 0, 1, 3, 4)
            .astype(BF16)
        )
        xkeys = ("x0", "x1")
        embkey = "embs"
    w1s = np.ascontiguousarray(
        np.asarray(W1).reshape(E, KC, 128, DC, 128).transpose(0, 2, 1, 3, 4).astype(BF16)
    )
    w2s = np.ascontiguousarray(
        np.asarray(W2).reshape(E, DC, 128, OUT).transpose(2, 0, 1, 3).astype(BF16)
    )
    wgs = np.ascontiguousarray(
        np.asarray(Wg).reshape(KC, 128, E).transpose(1, 0, 2).astype(BF16)
    )
    b1s = np.ascontiguousarray(
        np.asarray(b1).reshape(E, DC, 128).transpose(2, 0, 1).astype(np.float32)
    )
    b2s = np.ascontiguousarray(np.asarray(b2).T.astype(np.float32))
    bgs = np.ascontiguousarray(np.asarray(bg).reshape(E, 1).astype(np.float32))
    ivs = np.ascontiguousarray(
        (np.arange(VC)[None, :] * 128 + np.arange(128)[:, None]).astype(np.float32)
    )
    sels = np.ascontiguousarray(
        np.broadcast_to(np.eye(E, dtype=np.float32)[:, :, None], (E, E, 128)).astype(
            BF16
        )
    )
    shared = {
        embkey: embs, "w1s": w1s, "w2s": w2s, "wgs": wgs,
        "b1s": b1s, "b2s": b2s, "bgs": bgs, "sels": sels,
    }
    if not gather:
        shared["ivs"] = ivs
    return [{xkeys[0]: x0h[c], xkeys[1]: x1h[c], **shared} for c in range(ncores)]


def kernel(x, emb0, emb1, W1, b1, W2, b2, Wg, bg):
    global LAST_EXEC_NS
    nc = build_program()
    in_maps = marshal_inputs(x, emb0, emb1, W1, b1, W2, b2, Wg, bg)
    trace = os.environ.get("BASSMOE_TRACE", "0") == "1"
    res = run_bass_kernel_spmd(nc, in_maps, list(range(NCORES)), trace=trace)
    LAST_EXEC_NS = res.exec_time_ns
    out = np.empty((B, OUT), dtype=np.float32)
    for c in range(NCORES):
        out[c * BL : (c + 1) * BL, :] = res.results[c]["out"].T
    return out


# revision 55
# speedup vs baseline: 1.0912x; 1.0206x over previous
"""MoE model (embed -> gate -> 4 dense experts -> softmax combine) on 8 TRN2 cores.

Data-parallel: batch (65536 tokens) sharded 8192/core; embedding tables,
expert weights, and gating weights replicated on every core (SBUF-resident,
bf16). All on-chip activations are kept feature-major ("transposed") so that
every matmul consumes operands in their natural layout:

  e_T[f, t]   = one-hot(vocab) matmul against the embedding tables
  h_T[d, t]   = silu(W1[e].T-tiles @ e_T + b1)        (PSUM fp32, evac bf16)
  eo_T[o, t]  = W2[e].T-tiles @ h_T + b2              (PSUM fp32)
  logits[e,t] = Wg.T-tiles @ e_T + bg ; softmax via exp / sum (unnormalized
                weights combined first, one reciprocal row scale at the end)
  out_T[o, t] = (sum_e exp_e * eo_e) * recip          (DVE, fp32)

Output per core is [128, 8192] (feature-major); host transposes on unshard.
"""

import os
import numpy as np
import ml_dtypes

import concourse.bass as bass
import concourse.mybir as mybir
import concourse.tile as tile
from concourse.vector_clock import ScopedClock, VectorClock
from concourse.bass_utils import run_bass_kernel_spmd

BF16 = ml_dtypes.bfloat16

B = 65536
V = 512
D = 1024
IN = 2048
E = 4
OUT = 128
NCORES = 8
BL = B // NCORES          # tokens per core
ST = 512                  # tokens per supertile (max PSUM free dim, fp32)
NST = BL // ST            # supertiles per core
KC = IN // 128            # 16 feature chunks
DC = D // 128             # 8 hidden chunks
VC = V // 128             # 4 vocab chunks

LAST_EXEC_NS = None       # set when BASSMOE_TRACE=1


class _TC(tile.TileContext):
    """Unmodified TileContext; kept as a named subclass for clarity."""


def _legalize_waits(nc, max_waits=1):
    """This walrus build rejects instructions carrying more than ~1 sync-wait
    command ("Too many sync wait commands", CoreV2/V3GenImpl setupSyncWait).
    Hoist all but the last wait of every instruction onto single-wait NoOps
    placed immediately before it in the same engine's stream."""
    for f in nc.m.functions:
        for bb in f.blocks:
            insts = bb.instructions
            if not any(
                inst.sync_info is not None and len(inst.sync_info.on_wait) > max_waits
                for inst in insts
            ):
                continue
            new = []
            for inst in insts:
                si = inst.sync_info
                waits = list(si.on_wait) if si is not None else []
                if len(waits) > max_waits:
                    for w in waits[:-max_waits]:
                        nop = mybir.InstNoOp(
                            name=f"legw-{nc.next_id()}", ins=[], outs=[]
                        )
                        nop.engine = inst.engine
                        nop.sync_info = mybir.SyncInfo(on_wait=[w], on_update=[])
                        new.append(nop)
                    inst.sync_info = mybir.SyncInfo(
                        on_wait=waits[-max_waits:], on_update=list(si.on_update)
                    )
                new.append(inst)
            bb.instructions = new


def build_program(nst=NST, legalize=True, gather=False):
    dt = mybir.dt
    f32, bf16, f16 = dt.float32, dt.bfloat16, dt.float16
    AF = mybir.ActivationFunctionType
    ALU = mybir.AluOpType

    nc = bass.Bass()

    if gather:
        # token index tiles in dma_gather's wrapped layout: idx j at [j%16, j//16],
        # replicated across the 8 gpsimd cores (8 x 16 = 128 partitions)
        x0d = nc.dram_tensor("x0i", [nst, 128, ST // 16], dt.int16, kind="ExternalInput")
        x1d = nc.dram_tensor("x1i", [nst, 128, ST // 16], dt.int16, kind="ExternalInput")
        embd = nc.dram_tensor("embg", [2, V, D], bf16, kind="ExternalInput")
    else:
        x0d = nc.dram_tensor("x0", [nst, 1, ST], f16, kind="ExternalInput")
        x1d = nc.dram_tensor("x1", [nst, 1, ST], f16, kind="ExternalInput")
        embd = nc.dram_tensor("embs", [128, 2, VC, DC, 128], bf16, kind="ExternalInput")
    w1d = nc.dram_tensor("w1s", [E, 128, KC, DC, 128], bf16, kind="ExternalInput")
    w2d = nc.dram_tensor("w2s", [128, E, DC, OUT], bf16, kind="ExternalInput")
    wgd = nc.dram_tensor("wgs", [128, KC, E], bf16, kind="ExternalInput")
    b1d = nc.dram_tensor("b1s", [128, E, DC], f32, kind="ExternalInput")
    b2d = nc.dram_tensor("b2s", [128, E], f32, kind="ExternalInput")
    bgd = nc.dram_tensor("bgs", [E, 1], f32, kind="ExternalInput")
    if not gather:
        ivd = nc.dram_tensor("ivs", [128, VC], f32, kind="ExternalInput")
    seld = nc.dram_tensor("sels", [E, E, 128], bf16, kind="ExternalInput")
    outd = nc.dram_tensor("out", [128, nst * ST], f32, kind="ExternalOutput")

    with _TC(nc) as tc:
        with (
            tc.tile_pool(name="const", bufs=1) as cpool,
            tc.tile_pool(name="xt", bufs=2) as xpool,
            tc.tile_pool(name="mask", bufs=1) as mpool,
            tc.tile_pool(name="et", bufs=2 if gather else 1) as etpool,
            tc.tile_pool(name="hs", bufs=1) as hpool,
            tc.tile_pool(name="sm", bufs=1) as smpool,
            tc.tile_pool(name="gsc", bufs=1) as gspool,
            tc.tile_pool(name="sgp", bufs=2) as sgpool,
            tc.tile_pool(name="accp", bufs=1) as apool,
            tc.tile_pool(name="outp", bufs=2) as opool,
            tc.tile_pool(name="pmm", bufs=2, space="PSUM") as pmm,
            tc.tile_pool(name="peo", bufs=2, space="PSUM") as peo,
            tc.tile_pool(name="prb", bufs=2, space="PSUM") as prb,
            tc.tile_pool(name="pmisc", bufs=2, space="PSUM") as pmisc,
        ):
            # --- resident weights / constants (order = DMA priority: the
            # first supertile needs x/iv/emb/wg immediately, w1[e] at ~20us) ---
            if not gather:
                iv_sb = cpool.tile([128, VC], f32)
                nc.sync.dma_start(iv_sb[:], ivd[:])
                ones_f16 = cpool.tile([1, 128], f16)
                nc.vector.memset(ones_f16[:], 1.0)
                x00_sb = xpool.tile([1, ST], f16, tag="x0")
                nc.sync.dma_start(x00_sb[:], x0d[0])
                x10_sb = xpool.tile([1, ST], f16, tag="x1")
                nc.sync.dma_start(x10_sb[:], x1d[0])
                emb_sb = cpool.tile([128, 2, VC, DC, 128], bf16)
                nc.sync.dma_start(emb_sb[:], embd[:])
            wg_sb = cpool.tile([128, KC, E], bf16)
            nc.sync.dma_start(wg_sb[:], wgd[:])
            b1_sb = cpool.tile([128, E, DC], f32)
            nc.sync.dma_start(b1_sb[:], b1d[:])
            b2_sb = cpool.tile([128, E], f32)
            nc.sync.dma_start(b2_sb[:], b2d[:])
            bg_sb = cpool.tile([E, 1], f32)
            nc.sync.dma_start(bg_sb[:], bgd[:])
            sel_sb = cpool.tile([E, E, 128], bf16)
            nc.sync.dma_start(sel_sb[:], seld[:])
            w2_sb = cpool.tile([128, E, DC, OUT], bf16)
            nc.sync.dma_start(w2_sb[:], w2d[:])
            w1_sbs = []
            for e in range(E):
                t = cpool.tile([128, KC, DC, 128], bf16, tag=f"w1e{e}")
                nc.sync.dma_start(t[:], w1d[e])
                w1_sbs.append(t)

            ones4_bf = cpool.tile([E, 1], bf16)
            nc.vector.memset(ones4_bf[:], 1.0)
            ones128_bf = cpool.tile([1, 128], bf16)
            nc.vector.memset(ones128_bf[:], 1.0)

            if gather:
                from concourse import library_config

                nc.gpsimd.load_library(library_config.mlp)

            def build_masks(i, preloaded=None):
                """x-broadcast (K=1 matmul) + one-hot compares for supertile i."""
                if preloaded is None:
                    x0_sb = xpool.tile([1, ST], f16, tag="x0")
                    nc.sync.dma_start(x0_sb[:], x0d[i])
                    x1_sb = xpool.tile([1, ST], f16, tag="x1")
                    nc.sync.dma_start(x1_sb[:], x1d[i])
                else:
                    x0_sb, x1_sb = preloaded
                ms = []
                for tbl, xs in enumerate((x0_sb, x1_sb)):
                    p = pmisc.tile([128, ST], f32, tag="misc")
                    nc.tensor.matmul(p[:], ones_f16[:], xs[:])
                    row = []
                    for vc in range(VC):
                        m = mpool.tile([128, ST], bf16, tag=f"m{tbl}{vc}")
                        nc.vector.tensor_scalar(
                            m[:], p[:], iv_sb[:, vc : vc + 1], None, ALU.is_equal
                        )
                        row.append(m)
                    ms.append(row)
                return ms

            cur_masks = (
                None if gather else build_masks(0, preloaded=(x00_sb, x10_sb))
            )

            for i in range(nst):
                eT = etpool.tile([128, KC, ST], bf16, tag="eT")
                if gather:
                    # --- embedding lookup via transposing gather DMA ---
                    for tbl, xd in enumerate((x0d, x1d)):
                        xi = xpool.tile([128, ST // 16], dt.int16, tag=f"xi{tbl}")
                        nc.sync.dma_start(xi[:], xd[i])
                        nc.gpsimd.dma_gather(
                            out_ap=eT[:, tbl * DC : (tbl + 1) * DC, :],
                            in_ap=embd[tbl],
                            idxs_ap=xi[:],
                            num_idxs=ST,
                            num_idxs_reg=ST,
                            elem_size=D,
                            transpose=True,
                        )
                else:
                    # --- one-hot embedding matmul -> e_T ---
                    for tbl in range(2):
                        for dc in range(DC):
                            ps = pmm.tile([128, ST], f32, tag="mm")
                            for vc in range(VC):
                                nc.tensor.matmul(
                                    ps[:],
                                    emb_sb[:, tbl, vc, dc, :],
                                    cur_masks[tbl][vc][:],
                                    start=(vc == 0),
                                    stop=(vc == VC - 1),
                                )
                            nc.scalar.copy(eT[:, tbl * DC + dc, :], ps[:])

                # --- gating: logits -> exp -> sum -> reciprocal bcast ---
                lp = pmisc.tile([E, ST], f32, tag="misc")
                for kc in range(KC):
                    nc.tensor.matmul(
                        lp[:],
                        wg_sb[:, kc, :],
                        eT[:, kc, :],
                        start=(kc == 0),
                        stop=(kc == KC - 1),
                    )
                expt = smpool.tile([E, ST], bf16, tag="expt")
                nc.scalar.activation(expt[:], lp[:], AF.Exp, bias=bg_sb[:])
                sp = pmisc.tile([1, ST], f32, tag="misc")
                nc.tensor.matmul(sp[:], ones4_bf[:], expt[:])
                rec = smpool.tile([1, ST], f32, tag="rec")
                nc.vector.reciprocal(rec[:], sp[:])
                recb = smpool.tile([1, ST], bf16, tag="recb")
                nc.vector.tensor_copy(recb[:], rec[:])
                rbp = prb.tile([128, ST], f32, tag="rb")
                nc.tensor.matmul(rbp[:], ones128_bf[:], recb[:])

                # masks for the next supertile: the 2 tiny PE matmuls run now,
                # the DVE compares overlap with the expert phase below, so the
                # next e-build starts without a cross-engine stall
                if not gather and i + 1 < nst:
                    next_masks = build_masks(i + 1)

                # --- experts ---
                acc = apool.tile([128, ST], f32, tag="acc")
                for e in range(E):
                    hs = hpool.tile([128, DC, ST], bf16, tag="hs")
                    for dc in range(DC):
                        hp = pmm.tile([128, ST], f32, tag="mm")
                        for kc in range(KC):
                            nc.tensor.matmul(
                                hp[:],
                                w1_sbs[e][:, kc, dc, :],
                                eT[:, kc, :],
                                start=(kc == 0),
                                stop=(kc == KC - 1),
                            )
                        sg = sgpool.tile([128, ST], f32, tag="sg")
                        nc.scalar.activation(
                            sg[:], hp[:], AF.Sigmoid, bias=b1_sb[:, e, dc : dc + 1]
                        )
                        nc.vector.scalar_tensor_tensor(
                            hs[:, dc, :], hp[:], b1_sb[:, e, dc : dc + 1], sg[:],
                            ALU.add, ALU.mult,
                        )
                    eop = peo.tile([128, ST], f32, tag="eo")
                    for dc in range(DC):
                        nc.tensor.matmul(
                            eop[:],
                            w2_sb[:, e, dc, :],
                            hs[:, dc, :],
                            start=(dc == 0),
                            stop=(dc == DC - 1),
                        )
                    gp = pmisc.tile([128, ST], f32, tag="misc")
                    nc.tensor.matmul(gp[:], sel_sb[:, e, :], expt[:])
                    gs = gspool.tile([128, ST], f32, tag="gs")
                    nc.scalar.copy(gs[:], gp[:])
                    if e == 0:
                        nc.vector.scalar_tensor_tensor(
                            acc[:], eop[:], b2_sb[:, e : e + 1], gs[:], ALU.add, ALU.mult
                        )
                    else:
                        tmp = opool.tile([128, ST], f32, tag="outt")
                        nc.vector.scalar_tensor_tensor(
                            tmp[:], eop[:], b2_sb[:, e : e + 1], gs[:], ALU.add, ALU.mult
                        )
                        nc.vector.tensor_add(acc[:], acc[:], tmp[:])

                outt = opool.tile([128, ST], f32, tag="outt")
                nc.vector.tensor_tensor(outt[:], acc[:], rbp[:], ALU.mult)
                nc.sync.dma_start(outd[:, i * ST : (i + 1) * ST], outt[:])
                if not gather and i + 1 < nst:
                    cur_masks = next_masks

    if legalize:
        _legalize_waits(nc)
    # populate .instr bytes for extended-ISA instructions (library reload for
    # dma_gather) — raw Bass skips Bacc's codegen pass; walrus errors with
    # "ISA wrong length" on empty instr otherwise
    mybir.codegen_inst_isa_subclasses(nc)
    return nc


def marshal_inputs(
    x, emb0, emb1, W1, b1, W2, b2, Wg, bg, nst=NST, ncores=NCORES, gather=False
):
    """Host-side: cast/reshape full inputs into per-core in_maps."""
    n_tok = ncores * nst * ST
    if gather:
        # wrapped int16 index layout: token j of a supertile at [j%16, j//16],
        # tiled 8x across partitions for the 8 gpsimd cores
        def _wrap(col):
            w = (
                col[:n_tok].astype(np.int16).reshape(ncores, nst, ST // 16, 16)
                .transpose(0, 1, 3, 2)
            )
            return np.ascontiguousarray(np.tile(w, (1, 1, 8, 1)))

        x0h = _wrap(x[:, 0])
        x1h = _wrap(x[:, 1])
        embs = np.ascontiguousarray(np.stack([emb0, emb1]).astype(BF16))
        xkeys = ("x0i", "x1i")
        embkey = "embg"
    else:
        x0h = np.ascontiguousarray(
            x[:n_tok, 0].astype(np.float16).reshape(ncores, nst, 1, ST)
        )
        x1h = np.ascontiguousarray(
            x[:n_tok, 1].astype(np.float16).reshape(ncores, nst, 1, ST)
        )
        embs = np.ascontiguousarray(
            np.stack([emb0, emb1])
            .reshape(2, VC, 128, DC, 128)
            .transpose(2, 0, 1, 3, 4)
            .astype(BF16)
        )
        xkeys = ("x0", "x1")
        embkey = "embs"
    w1s = np.ascontiguousarray(
        np.asarray(W1).reshape(E, KC, 128, DC, 128).transpose(0, 2, 1, 3, 4).astype(BF16)
    )
    w2s = np.ascontiguousarray(
        np.asarray(W2).reshape(E, DC, 128, OUT).transpose(2, 0, 1, 3).astype(BF16)
    )
    wgs = np.ascontiguousarray(
        np.asarray(Wg).reshape(KC, 128, E).transpose(1, 0, 2).astype(BF16)
    )
    b1s = np.ascontiguousarray(
        np.asarray(b1).reshape(E, DC, 128).transpose(2, 0, 1).astype(np.float32)
    )
    b2s = np.ascontiguousarray(np.asarray(b2).T.astype(np.float32))
    bgs = np.ascontiguousarray(np.asarray(bg).reshape(E, 1).astype(np.float32))
    ivs = np.ascontiguousarray(
        (np.arange(VC)[None, :] * 128 + np.arange(128)[:, None]).astype(np.float32)
    )
    sels = np.ascontiguousarray(
        np.broadcast_to(np.eye(E, dtype=np.float32)[:, :, None], (E, E, 128)).astype(
            BF16
        )
    )
    shared = {
        embkey: embs, "w1s": w1s, "w2s": w2s, "wgs": wgs,
        "b1s": b1s, "b2s": b2s, "bgs": bgs, "sels": sels,
    }
    if not gather:
        shared["ivs"] = ivs
    return [{xkeys[0]: x0h[c], xkeys[1]: x1h[c], **shared} for c in range(ncores)]


def kernel(x, emb0, emb1, W1, b1, W2, b2, Wg, bg):
    global LAST_EXEC_NS
    nc = build_program()
    in_maps = marshal_inputs(x, emb0, emb1, W1, b1, W2, b2, Wg, bg)
    trace = os.environ.get("BASSMOE_TRACE", "0") == "1"
    res = run_bass_kernel_spmd(nc, in_maps, list(range(NCORES)), trace=trace)
    LAST_EXEC_NS = res.exec_time_ns
    out = np.empty((B, OUT), dtype=np.float32)
    for c in range(NCORES):
        out[c * BL : (c + 1) * BL, :] = res.results[c]["out"].T
    return out


# revision 56
# speedup vs baseline: 1.1510x; 1.0548x over previous
"""MoE model (embed -> gate -> 4 dense experts -> softmax combine) on 8 TRN2 cores.

Data-parallel: batch (65536 tokens) sharded 8192/core; expert/gating weights
replicated on every core (SBUF-resident, bf16). All on-chip activations are
kept feature-major ("transposed") so that every matmul consumes operands in
their natural layout:

  e_T[f, t]   = embedding lookup, feature-major.  Hybrid strategy: table0 via
                a transposing gather DMA (runs on the otherwise-idle GpSimd
                SWDGE path), table1 via a one-hot-mask matmul on the PE.
  h_T[d, t]   = silu(W1[e].T-tiles @ e_T + b1)       (PSUM fp32, evac bf16)
  eo_T[o, t]  = W2[e].T-tiles @ h_T + b2             (PSUM fp32)
  logits[e,t] = Wg.T-tiles @ e_T + bg ; softmax via exp / sum (unnormalized
                weights combined first, one reciprocal row scale at the end)
  out_T[o, t] = (sum_e exp_e * eo_e) * recip         (DVE, fp32)

Output per core is [128, 8192] (feature-major); host transposes on unshard.

bf16 inputs with fp32 PSUM accumulation: end-to-end relative error vs the
fp32 reference is ~0.5%.
"""

import os
import numpy as np
import ml_dtypes

import concourse.bass as bass
import concourse.mybir as mybir
import concourse.tile as tile
from concourse.bass_utils import run_bass_kernel_spmd

BF16 = ml_dtypes.bfloat16

B = 65536
V = 512
D = 1024
IN = 2048
E = 4
OUT = 128
NCORES = 8
BL = B // NCORES          # tokens per core
ST = 512                  # tokens per supertile (max PSUM free dim, fp32)
NST = BL // ST            # supertiles per core
KC = IN // 128            # 16 feature chunks
DC = D // 128             # 8 hidden chunks
VC = V // 128             # 4 vocab chunks

LAST_EXEC_NS = None       # set when BASSMOE_TRACE=1


def _legalize_waits(nc, max_waits=1):
    """This walrus build rejects instructions carrying more than ~1 sync-wait
    command ("Too many sync wait commands", CoreV2/V3GenImpl setupSyncWait).
    Hoist all but the last wait of every instruction onto single-wait NoOps
    placed immediately before it in the same engine's stream."""
    for f in nc.m.functions:
        for bb in f.blocks:
            insts = bb.instructions
            if not any(
                inst.sync_info is not None and len(inst.sync_info.on_wait) > max_waits
                for inst in insts
            ):
                continue
            new = []
            for inst in insts:
                si = inst.sync_info
                waits = list(si.on_wait) if si is not None else []
                if len(waits) > max_waits:
                    for w in waits[:-max_waits]:
                        nop = mybir.InstNoOp(
                            name=f"legw-{nc.next_id()}", ins=[], outs=[]
                        )
                        nop.engine = inst.engine
                        nop.sync_info = mybir.SyncInfo(on_wait=[w], on_update=[])
                        new.append(nop)
                    inst.sync_info = mybir.SyncInfo(
                        on_wait=waits[-max_waits:], on_update=list(si.on_update)
                    )
                new.append(inst)
            bb.instructions = new


def build_program(nst=NST, legalize=True, hybrid=True):
    dt = mybir.dt
    f32, bf16, f16 = dt.float32, dt.bfloat16, dt.float16
    AF = mybir.ActivationFunctionType
    ALU = mybir.AluOpType

    nc = bass.Bass()

    if hybrid:
        # table0 token indices in dma_gather's wrapped layout: idx j at
        # [j%16, j//16], replicated across the 8 gpsimd cores
        x0d = nc.dram_tensor(
            "x0i", [nst, 128, ST // 16], dt.int16, kind="ExternalInput"
        )
        emb0d = nc.dram_tensor("embg", [V, D], bf16, kind="ExternalInput")
        # table1 one-hot operands: emb1 as [vocab-chunk, d-chunk] lhsT tiles
        embd = nc.dram_tensor("embs", [128, 1, VC, DC, 128], bf16, kind="ExternalInput")
    else:
        x0d = nc.dram_tensor("x0", [nst, 1, ST], f16, kind="ExternalInput")
        embd = nc.dram_tensor("embs", [128, 2, VC, DC, 128], bf16, kind="ExternalInput")
    x1d = nc.dram_tensor("x1", [nst, 1, ST], f16, kind="ExternalInput")
    w1d = nc.dram_tensor("w1s", [E, 128, KC, DC, 128], bf16, kind="ExternalInput")
    w2d = nc.dram_tensor("w2s", [128, E, DC, OUT], bf16, kind="ExternalInput")
    wgd = nc.dram_tensor("wgs", [128, KC, E], bf16, kind="ExternalInput")
    b1d = nc.dram_tensor("b1s", [128, E, DC], f32, kind="ExternalInput")
    b2d = nc.dram_tensor("b2s", [128, E], f32, kind="ExternalInput")
    bgd = nc.dram_tensor("bgs", [E, 1], f32, kind="ExternalInput")
    ivd = nc.dram_tensor("ivs", [128, VC], f32, kind="ExternalInput")
    seld = nc.dram_tensor("sels", [E, E, 128], bf16, kind="ExternalInput")
    outd = nc.dram_tensor("out", [128, nst * ST], f32, kind="ExternalOutput")

    n_onehot_tables = 1 if hybrid else 2

    with tile.TileContext(nc) as tc:
        with (
            tc.tile_pool(name="const", bufs=1) as cpool,
            tc.tile_pool(name="xt", bufs=2) as xpool,
            tc.tile_pool(name="mask", bufs=1) as mpool,
            tc.tile_pool(name="etg", bufs=2) as etgpool,
            tc.tile_pool(name="et", bufs=1) as etpool,
            tc.tile_pool(name="hs", bufs=1) as hpool,
            tc.tile_pool(name="sm", bufs=1) as smpool,
            tc.tile_pool(name="gsc", bufs=1) as gspool,
            tc.tile_pool(name="sgp", bufs=2) as sgpool,
            tc.tile_pool(name="accp", bufs=1) as apool,
            tc.tile_pool(name="outp", bufs=2) as opool,
            tc.tile_pool(name="pmm", bufs=2, space="PSUM") as pmm,
            tc.tile_pool(name="peo", bufs=2, space="PSUM") as peo,
            tc.tile_pool(name="prb", bufs=2, space="PSUM") as prb,
            tc.tile_pool(name="pmisc", bufs=2, space="PSUM") as pmisc,
        ):
            # --- prologue: things supertile 0 needs first ---
            iv_sb = cpool.tile([128, VC], f32)
            nc.sync.dma_start(iv_sb[:], ivd[:])
            ones_f16 = cpool.tile([1, 128], f16)
            nc.vector.memset(ones_f16[:], 1.0)
            if hybrid:
                from concourse import library_config

                nc.gpsimd.load_library(library_config.mlp)

                def issue_gather(i):
                    """table0 embedding rows for supertile i -> feature-major
                    e_T chunk tile, via the GpSimd transposing gather DMA."""
                    xi = xpool.tile([128, ST // 16], dt.int16, tag="xi")
                    nc.sync.dma_start(xi[:], x0d[i])
                    etg = etgpool.tile([128, DC, ST], bf16, tag="eTg")
                    nc.gpsimd.dma_gather(
                        out_ap=etg[:],
                        in_ap=emb0d[:],
                        idxs_ap=xi[:],
                        num_idxs=ST,
                        num_idxs_reg=ST,
                        elem_size=D,
                        transpose=True,
                    )
                    return etg

                cur_etg = issue_gather(0)
            else:
                x00_sb = xpool.tile([1, ST], f16, tag="x0")
                nc.sync.dma_start(x00_sb[:], x0d[0])
            x10_sb = xpool.tile([1, ST], f16, tag="x1")
            nc.sync.dma_start(x10_sb[:], x1d[0])
            emb_sb = cpool.tile([128, n_onehot_tables, VC, DC, 128], bf16)
            nc.sync.dma_start(emb_sb[:], embd[:])

            # --- resident weights (DMA queue order = when they are needed) ---
            wg_sb = cpool.tile([128, KC, E], bf16)
            nc.sync.dma_start(wg_sb[:], wgd[:])
            b1_sb = cpool.tile([128, E, DC], f32)
            nc.sync.dma_start(b1_sb[:], b1d[:])
            b2_sb = cpool.tile([128, E], f32)
            nc.sync.dma_start(b2_sb[:], b2d[:])
            bg_sb = cpool.tile([E, 1], f32)
            nc.sync.dma_start(bg_sb[:], bgd[:])
            sel_sb = cpool.tile([E, E, 128], bf16)
            nc.sync.dma_start(sel_sb[:], seld[:])
            w2_sb = cpool.tile([128, E, DC, OUT], bf16)
            nc.sync.dma_start(w2_sb[:], w2d[:])
            w1_sbs = []
            for e in range(E):
                t = cpool.tile([128, KC, DC, 128], bf16, tag=f"w1e{e}")
                nc.sync.dma_start(t[:], w1d[e])
                w1_sbs.append(t)

            ones4_bf = cpool.tile([E, 1], bf16)
            nc.vector.memset(ones4_bf[:], 1.0)
            ones128_bf = cpool.tile([1, 128], bf16)
            nc.vector.memset(ones128_bf[:], 1.0)

            def build_masks(i, preloaded=None):
                """x-broadcast (K=1 matmul) + one-hot compares for the
                one-hot-embedded tables of supertile i."""
                xs_tiles = []
                if hybrid:
                    if preloaded is None:
                        x1_sb = xpool.tile([1, ST], f16, tag="x1")
                        nc.sync.dma_start(x1_sb[:], x1d[i])
                        xs_tiles = [x1_sb]
                    else:
                        xs_tiles = [preloaded[0]]
                else:
                    if preloaded is None:
                        x0_sb = xpool.tile([1, ST], f16, tag="x0")
                        nc.sync.dma_start(x0_sb[:], x0d[i])
                        x1_sb = xpool.tile([1, ST], f16, tag="x1")
                        nc.sync.dma_start(x1_sb[:], x1d[i])
                        xs_tiles = [x0_sb, x1_sb]
                    else:
                        xs_tiles = list(preloaded)
                ms = []
                for tbl, xs in enumerate(xs_tiles):
                    p = pmisc.tile([128, ST], f32, tag="misc")
                    nc.tensor.matmul(p[:], ones_f16[:], xs[:])
                    row = []
                    for vc in range(VC):
                        m = mpool.tile([128, ST], bf16, tag=f"m{tbl}{vc}")
                        nc.vector.tensor_scalar(
                            m[:], p[:], iv_sb[:, vc : vc + 1], None, ALU.is_equal
                        )
                        row.append(m)
                    ms.append(row)
                return ms

            if hybrid:
                cur_masks = build_masks(0, preloaded=(x10_sb,))
            else:
                cur_masks = build_masks(0, preloaded=(x00_sb, x10_sb))

            for i in range(nst):
                # --- one-hot embedding matmul -> e_T (one-hot tables) ---
                eT = etpool.tile([128, n_onehot_tables * DC, ST], bf16, tag="eT")
                for tbl in range(n_onehot_tables):
                    for dc in range(DC):
                        ps = pmm.tile([128, ST], f32, tag="mm")
                        for vc in range(VC):
                            nc.tensor.matmul(
                                ps[:],
                                emb_sb[:, tbl, vc, dc, :],
                                cur_masks[tbl][vc][:],
                                start=(vc == 0),
                                stop=(vc == VC - 1),
                            )
                        nc.scalar.copy(eT[:, tbl * DC + dc, :], ps[:])

                if hybrid:
                    etg = cur_etg

                    def eT_chunk(kc):
                        return etg[:, kc, :] if kc < DC else eT[:, kc - DC, :]
                else:

                    def eT_chunk(kc):
                        return eT[:, kc, :]

                # --- gating: logits -> exp -> sum -> reciprocal bcast ---
                lp = pmisc.tile([E, ST], f32, tag="misc")
                for kc in range(KC):
                    nc.tensor.matmul(
                        lp[:],
                        wg_sb[:, kc, :],
                        eT_chunk(kc),
                        start=(kc == 0),
                        stop=(kc == KC - 1),
                    )
                expt = smpool.tile([E, ST], bf16, tag="expt")
                nc.scalar.activation(expt[:], lp[:], AF.Exp, bias=bg_sb[:])
                sp = pmisc.tile([1, ST], f32, tag="misc")
                nc.tensor.matmul(sp[:], ones4_bf[:], expt[:])
                rec = smpool.tile([1, ST], f32, tag="rec")
                nc.vector.reciprocal(rec[:], sp[:])
                recb = smpool.tile([1, ST], bf16, tag="recb")
                nc.vector.tensor_copy(recb[:], rec[:])
                rbp = prb.tile([128, ST], f32, tag="rb")
                nc.tensor.matmul(rbp[:], ones128_bf[:], recb[:])

                # prefetch next supertile's embeddings: gather DMA + mask
                # compares overlap with the expert phase below
                if i + 1 < nst:
                    if hybrid:
                        next_etg = issue_gather(i + 1)
                    next_masks = build_masks(i + 1)

                # --- experts ---
                acc = apool.tile([128, ST], f32, tag="acc")
                for e in range(E):
                    hs = hpool.tile([128, DC, ST], bf16, tag="hs")
                    for dc in range(DC):
                        hp = pmm.tile([128, ST], f32, tag="mm")
                        for kc in range(KC):
                            nc.tensor.matmul(
                                hp[:],
                                w1_sbs[e][:, kc, dc, :],
                                eT_chunk(kc),
                                start=(kc == 0),
                                stop=(kc == KC - 1),
                            )
                        sg = sgpool.tile([128, ST], f32, tag="sg")
                        nc.scalar.activation(
                            sg[:], hp[:], AF.Sigmoid, bias=b1_sb[:, e, dc : dc + 1]
                        )
                        nc.vector.scalar_tensor_tensor(
                            hs[:, dc, :], hp[:], b1_sb[:, e, dc : dc + 1], sg[:],
                            ALU.add, ALU.mult,
                        )
                    eop = peo.tile([128, ST], f32, tag="eo")
                    for dc in range(DC):
                        nc.tensor.matmul(
                            eop[:],
                            w2_sb[:, e, dc, :],
                            hs[:, dc, :],
                            start=(dc == 0),
                            stop=(dc == DC - 1),
                        )
                    gp = pmisc.tile([128, ST], f32, tag="misc")
                    nc.tensor.matmul(gp[:], sel_sb[:, e, :], expt[:])
                    gs = gspool.tile([128, ST], f32, tag="gs")
                    nc.scalar.copy(gs[:], gp[:])
                    if e == 0:
                        nc.vector.scalar_tensor_tensor(
                            acc[:], eop[:], b2_sb[:, e : e + 1], gs[:],
                            ALU.add, ALU.mult,
                        )
                    else:
                        tmp = opool.tile([128, ST], f32, tag="outt")
                        nc.vector.scalar_tensor_tensor(
                            tmp[:], eop[:], b2_sb[:, e : e + 1], gs[:],
                            ALU.add, ALU.mult,
                        )
                        nc.vector.tensor_add(acc[:], acc[:], tmp[:])

                outt = opool.tile([128, ST], f32, tag="outt")
                nc.vector.tensor_tensor(outt[:], acc[:], rbp[:], ALU.mult)
                nc.sync.dma_start(outd[:, i * ST : (i + 1) * ST], outt[:])
                if i + 1 < nst:
                    cur_masks = next_masks
                    if hybrid:
                        cur_etg = next_etg

    if legalize:
        _legalize_waits(nc)
    # populate .instr bytes for extended-ISA instructions (library reload for
    # dma_gather) — raw Bass skips Bacc's codegen pass; walrus errors with
    # "ISA wrong length" on empty instr otherwise
    mybir.codegen_inst_isa_subclasses(nc)
    return nc


def marshal_inputs(
    x, emb0, emb1, W1, b1, W2, b2, Wg, bg, nst=NST, ncores=NCORES, hybrid=True
):
    """Host-side: cast/reshape full inputs into per-core in_maps."""
    n_tok = ncores * nst * ST

    def _wrap_idx(col):
        # dma_gather wrapped layout, tiled 8x across partitions (8 Q7 cores)
        w = (
            col[:n_tok].astype(np.int16).reshape(ncores, nst, ST // 16, 16)
            .transpose(0, 1, 3, 2)
        )
        return np.ascontiguousarray(np.tile(w, (1, 1, 8, 1)))

    def _f16_rows(col):
        return np.ascontiguousarray(
            col[:n_tok].astype(np.float16).reshape(ncores, nst, 1, ST)
        )

    def _onehot_tables(tables):
        return np.ascontiguousarray(
            np.stack(tables)
            .reshape(len(tables), VC, 128, DC, 128)
            .transpose(2, 0, 1, 3, 4)
            .astype(BF16)
        )

    if hybrid:
        x0h = _wrap_idx(x[:, 0])
        embg = np.ascontiguousarray(np.asarray(emb0).astype(BF16))
        embs = _onehot_tables([emb1])
        x0key = "x0i"
    else:
        x0h = _f16_rows(x[:, 0])
        embs = _onehot_tables([emb0, emb1])
        x0key = "x0"
    x1h = _f16_rows(x[:, 1])

    w1s = np.ascontiguousarray(
        np.asarray(W1).reshape(E, KC, 128, DC, 128).transpose(0, 2, 1, 3, 4).astype(BF16)
    )
    w2s = np.ascontiguousarray(
        np.asarray(W2).reshape(E, DC, 128, OUT).transpose(2, 0, 1, 3).astype(BF16)
    )
    wgs = np.ascontiguousarray(
        np.asarray(Wg).reshape(KC, 128, E).transpose(1, 0, 2).astype(BF16)
    )
    b1s = np.ascontiguousarray(
        np.asarray(b1).reshape(E, DC, 128).transpose(2, 0, 1).astype(np.float32)
    )
    b2s = np.ascontiguousarray(np.asarray(b2).T.astype(np.float32))
    bgs = np.ascontiguousarray(np.asarray(bg).reshape(E, 1).astype(np.float32))
    ivs = np.ascontiguousarray(
        (np.arange(VC)[None, :] * 128 + np.arange(128)[:, None]).astype(np.float32)
    )
    sels = np.ascontiguousarray(
        np.broadcast_to(np.eye(E, dtype=np.float32)[:, :, None], (E, E, 128)).astype(
            BF16
        )
    )
    shared = {
        "embs": embs, "w1s": w1s, "w2s": w2s, "wgs": wgs,
        "b1s": b1s, "b2s": b2s, "bgs": bgs, "ivs": ivs, "sels": sels,
    }
    if hybrid:
        shared["embg"] = embg
    return [{x0key: x0h[c], "x1": x1h[c], **shared} for c in range(ncores)]


def kernel(x, emb0, emb1, W1, b1, W2, b2, Wg, bg):
    global LAST_EXEC_NS
    nc = build_program()
    in_maps = marshal_inputs(x, emb0, emb1, W1, b1, W2, b2, Wg, bg)
    trace = os.environ.get("BASSMOE_TRACE", "0") == "1"
    res = run_bass_kernel_spmd(nc, in_maps, list(range(NCORES)), trace=trace)
    LAST_EXEC_NS = res.exec_time_ns
    out = np.empty((B, OUT), dtype=np.float32)
    for c in range(NCORES):
        out[c * BL : (c + 1) * BL, :] = res.results[c]["out"].T
    return out


# revision 57
# speedup vs baseline: 1.2131x; 1.0539x over previous
"""MoE model (embed -> gate -> 4 dense experts -> softmax combine) on 8 TRN2 cores.

Data-parallel: batch (65536 tokens) sharded 8192/core; expert/gating weights
replicated on every core (SBUF-resident, bf16). All on-chip activations are
kept feature-major ("transposed") so that every matmul consumes operands in
their natural layout:

  e_T[f, t]   = embedding lookup, feature-major, via transposing gather DMAs
                issued one supertile ahead on the otherwise-idle GpSimd SWDGE
                path (fallback: one-hot-mask matmul on the PE).
  h_T[d, t]   = silu(W1[e].T-tiles @ e_T + b1)       (PSUM fp32, evac bf16)
  eo_T[o, t]  = W2[e].T-tiles @ h_T + b2             (PSUM fp32)
  logits[e,t] = Wg.T-tiles @ e_T + bg ; softmax via exp / sum (unnormalized
                weights combined first, one reciprocal row scale at the end)
  out_T[o, t] = (sum_e exp_e * eo_e) * recip         (DVE, fp32)

Output per core is [128, 8192] (feature-major); host transposes on unshard.

bf16 inputs with fp32 PSUM accumulation: end-to-end relative error vs the
fp32 reference is ~0.5%.
"""

import os
import numpy as np
import ml_dtypes

import concourse.bass as bass
import concourse.mybir as mybir
import concourse.tile as tile
from concourse.bass_utils import run_bass_kernel_spmd

BF16 = ml_dtypes.bfloat16

B = 65536
V = 512
D = 1024
IN = 2048
E = 4
OUT = 128
NCORES = 8
BL = B // NCORES          # tokens per core
ST = 512                  # tokens per supertile (max PSUM free dim, fp32)
NST = BL // ST            # supertiles per core
KC = IN // 128            # 16 feature chunks
DC = D // 128             # 8 hidden chunks
VC = V // 128             # 4 vocab chunks

LAST_EXEC_NS = None       # set when BASSMOE_TRACE=1


def _legalize_waits(nc, max_waits=1):
    """This walrus build rejects instructions carrying more than ~1 sync-wait
    command ("Too many sync wait commands", CoreV2/V3GenImpl setupSyncWait).
    Hoist all but the last wait of every instruction onto single-wait NoOps
    placed immediately before it in the same engine's stream."""
    for f in nc.m.functions:
        for bb in f.blocks:
            insts = bb.instructions
            if not any(
                inst.sync_info is not None and len(inst.sync_info.on_wait) > max_waits
                for inst in insts
            ):
                continue
            new = []
            for inst in insts:
                si = inst.sync_info
                waits = list(si.on_wait) if si is not None else []
                if len(waits) > max_waits:
                    for w in waits[:-max_waits]:
                        nop = mybir.InstNoOp(
                            name=f"legw-{nc.next_id()}", ins=[], outs=[]
                        )
                        nop.engine = inst.engine
                        nop.sync_info = mybir.SyncInfo(on_wait=[w], on_update=[])
                        new.append(nop)
                    inst.sync_info = mybir.SyncInfo(
                        on_wait=waits[-max_waits:], on_update=list(si.on_update)
                    )
                new.append(inst)
            bb.instructions = new


def build_program(nst=NST, legalize=True, n_gather=2):
    """n_gather: how many of the 2 embedding tables use the gather-DMA path
    (the rest use the one-hot matmul path)."""
    dt = mybir.dt
    f32, bf16, f16 = dt.float32, dt.bfloat16, dt.float16
    AF = mybir.ActivationFunctionType
    ALU = mybir.AluOpType

    gathered = [t < n_gather for t in range(2)]
    n_onehot = 2 - n_gather

    nc = bass.Bass()

    xd = [None, None]
    for t in range(2):
        if gathered[t]:
            # wrapped gather-idx layout: idx j at [j%16, j//16], replicated
            # across the 8 gpsimd cores
            xd[t] = nc.dram_tensor(
                f"x{t}i", [nst, 128, ST // 16], dt.int16, kind="ExternalInput"
            )
        else:
            xd[t] = nc.dram_tensor(
                f"x{t}", [nst, 1, ST], f16, kind="ExternalInput"
            )
    if n_gather:
        embgd = nc.dram_tensor("embg", [n_gather, V, D], bf16, kind="ExternalInput")
    if n_onehot:
        embd = nc.dram_tensor(
            "embs", [128, n_onehot, VC, DC, 128], bf16, kind="ExternalInput"
        )
        ivd = nc.dram_tensor("ivs", [128, VC], f32, kind="ExternalInput")
    w1d = nc.dram_tensor("w1s", [E, 128, KC, DC, 128], bf16, kind="ExternalInput")
    w2d = nc.dram_tensor("w2s", [128, E, DC, OUT], bf16, kind="ExternalInput")
    wgd = nc.dram_tensor("wgs", [128, KC, E], bf16, kind="ExternalInput")
    b1d = nc.dram_tensor("b1s", [128, E, DC], f32, kind="ExternalInput")
    b2d = nc.dram_tensor("b2s", [128, E], f32, kind="ExternalInput")
    bgd = nc.dram_tensor("bgs", [E, 1], f32, kind="ExternalInput")
    seld = nc.dram_tensor("sels", [E, E, 128], bf16, kind="ExternalInput")
    outd = nc.dram_tensor("out", [128, nst * ST], f32, kind="ExternalOutput")

    with tile.TileContext(nc) as tc:
        with (
            tc.tile_pool(name="const", bufs=1) as cpool,
            tc.tile_pool(name="xt", bufs=2) as xpool,
            tc.tile_pool(name="mask", bufs=1) as mpool,
            tc.tile_pool(name="etg", bufs=2) as etgpool,
            tc.tile_pool(name="et", bufs=1) as etpool,
            tc.tile_pool(name="hs", bufs=1) as hpool,
            tc.tile_pool(name="sm", bufs=1) as smpool,
            tc.tile_pool(name="gsc", bufs=1) as gspool,
            tc.tile_pool(name="sgp", bufs=2) as sgpool,
            tc.tile_pool(name="accp", bufs=1) as apool,
            tc.tile_pool(name="outp", bufs=2) as opool,
            tc.tile_pool(name="pmm", bufs=2, space="PSUM") as pmm,
            tc.tile_pool(name="peo", bufs=2, space="PSUM") as peo,
            tc.tile_pool(name="prb", bufs=2, space="PSUM") as prb,
            tc.tile_pool(name="pmisc", bufs=2, space="PSUM") as pmisc,
        ):
            # --- prologue: supertile 0's embedding inputs first ---
            if n_gather:
                from concourse import library_config

                nc.gpsimd.load_library(library_config.mlp)

                def issue_gather(i, t):
                    """table t embedding rows for supertile i -> feature-major
                    e_T chunk tile, via the GpSimd transposing gather DMA."""
                    xi = xpool.tile([128, ST // 16], dt.int16, tag=f"xi{t}")
                    nc.sync.dma_start(xi[:], xd[t][i])
                    etg = etgpool.tile([128, DC, ST], bf16, tag=f"eTg{t}")
                    nc.gpsimd.dma_gather(
                        out_ap=etg[:],
                        in_ap=embgd[t],
                        idxs_ap=xi[:],
                        num_idxs=ST,
                        num_idxs_reg=ST,
                        elem_size=D,
                        transpose=True,
                    )
                    return etg

            if n_onehot:
                iv_sb = cpool.tile([128, VC], f32)
                nc.sync.dma_start(iv_sb[:], ivd[:])
                ones_f16 = cpool.tile([1, 128], f16)
                nc.vector.memset(ones_f16[:], 1.0)
                x0_pre = []
                for t in range(2):
                    if not gathered[t]:
                        xs = xpool.tile([1, ST], f16, tag=f"x{t}")
                        nc.sync.dma_start(xs[:], xd[t][0])
                        x0_pre.append(xs)
                emb_sb = cpool.tile([128, n_onehot, VC, DC, 128], bf16)
                nc.sync.dma_start(emb_sb[:], embd[:])

            cur_etg = [issue_gather(0, t) if gathered[t] else None for t in range(2)]

            # --- resident weights (DMA queue order = when they are needed) ---
            wg_sb = cpool.tile([128, KC, E], bf16)
            nc.sync.dma_start(wg_sb[:], wgd[:])
            b1_sb = cpool.tile([128, E, DC], f32)
            nc.sync.dma_start(b1_sb[:], b1d[:])
            b2_sb = cpool.tile([128, E], f32)
            nc.sync.dma_start(b2_sb[:], b2d[:])
            bg_sb = cpool.tile([E, 1], f32)
            nc.sync.dma_start(bg_sb[:], bgd[:])
            sel_sb = cpool.tile([E, E, 128], bf16)
            nc.sync.dma_start(sel_sb[:], seld[:])
            w2_sb = cpool.tile([128, E, DC, OUT], bf16)
            nc.sync.dma_start(w2_sb[:], w2d[:])
            w1_sbs = []
            for e in range(E):
                t = cpool.tile([128, KC, DC, 128], bf16, tag=f"w1e{e}")
                nc.sync.dma_start(t[:], w1d[e])
                w1_sbs.append(t)

            ones4_bf = cpool.tile([E, 1], bf16)
            nc.vector.memset(ones4_bf[:], 1.0)
            ones128_bf = cpool.tile([1, 128], bf16)
            nc.vector.memset(ones128_bf[:], 1.0)

            def build_masks(i, preloaded=None):
                """x-broadcast (K=1 matmul) + one-hot compares for the
                one-hot-embedded tables of supertile i."""
                ms = {}
                pi = 0
                for t in range(2):
                    if gathered[t]:
                        continue
                    if preloaded is None:
                        xs = xpool.tile([1, ST], f16, tag=f"x{t}")
                        nc.sync.dma_start(xs[:], xd[t][i])
                    else:
                        xs = preloaded[pi]
                        pi += 1
                    p = pmisc.tile([128, ST], f32, tag="misc")
                    nc.tensor.matmul(p[:], ones_f16[:], xs[:])
                    row = []
                    for vc in range(VC):
                        m = mpool.tile([128, ST], bf16, tag=f"m{t}{vc}")
                        nc.vector.tensor_scalar(
                            m[:], p[:], iv_sb[:, vc : vc + 1], None, ALU.is_equal
                        )
                        row.append(m)
                    ms[t] = row
                return ms

            cur_masks = build_masks(0, preloaded=x0_pre) if n_onehot else {}

            for i in range(nst):
                # --- one-hot embedding matmul -> e_T (one-hot tables) ---
                if n_onehot:
                    eT = etpool.tile([128, n_onehot, DC, ST], bf16, tag="eT")
                    oh = 0
                    for t in range(2):
                        if gathered[t]:
                            continue
                        for dc in range(DC):
                            ps = pmm.tile([128, ST], f32, tag="mm")
                            for vc in range(VC):
                                nc.tensor.matmul(
                                    ps[:],
                                    emb_sb[:, oh, vc, dc, :],
                                    cur_masks[t][vc][:],
                                    start=(vc == 0),
                                    stop=(vc == VC - 1),
                                )
                            nc.scalar.copy(eT[:, oh, dc, :], ps[:])
                        oh += 1

                oh_index = {}
                oh = 0
                for t in range(2):
                    if not gathered[t]:
                        oh_index[t] = oh
                        oh += 1

                def eT_chunk(kc):
                    t, dc = kc // DC, kc % DC
                    if gathered[t]:
                        return cur_etg[t][:, dc, :]
                    return eT[:, oh_index[t], dc, :]

                # --- gating: logits -> exp -> sum -> reciprocal bcast ---
                lp = pmisc.tile([E, ST], f32, tag="misc")
                for kc in range(KC):
                    nc.tensor.matmul(
                        lp[:],
                        wg_sb[:, kc, :],
                        eT_chunk(kc),
                        start=(kc == 0),
                        stop=(kc == KC - 1),
                    )
                expt = smpool.tile([E, ST], bf16, tag="expt")
                nc.scalar.activation(expt[:], lp[:], AF.Exp, bias=bg_sb[:])
                sp = pmisc.tile([1, ST], f32, tag="misc")
                nc.tensor.matmul(sp[:], ones4_bf[:], expt[:])
                rec = smpool.tile([1, ST], f32, tag="rec")
                nc.vector.reciprocal(rec[:], sp[:])
                recb = smpool.tile([1, ST], bf16, tag="recb")
                nc.vector.tensor_copy(recb[:], rec[:])
                rbp = prb.tile([128, ST], f32, tag="rb")
                nc.tensor.matmul(rbp[:], ones128_bf[:], recb[:])

                # prefetch next supertile's embeddings: gather DMAs + mask
                # compares overlap with the expert phase below
                next_etg = [None, None]
                if i + 1 < nst:
                    for t in range(2):
                        if gathered[t]:
                            next_etg[t] = issue_gather(i + 1, t)
                    next_masks = build_masks(i + 1) if n_onehot else {}

                # --- experts ---
                acc = apool.tile([128, ST], f32, tag="acc")
                for e in range(E):
                    hs = hpool.tile([128, DC, ST], bf16, tag="hs")
                    for dc in range(DC):
                        hp = pmm.tile([128, ST], f32, tag="mm")
                        for kc in range(KC):
                            nc.tensor.matmul(
                                hp[:],
                                w1_sbs[e][:, kc, dc, :],
                                eT_chunk(kc),
                                start=(kc == 0),
                                stop=(kc == KC - 1),
                            )
                        sg = sgpool.tile([128, ST], f32, tag="sg")
                        nc.scalar.activation(
                            sg[:], hp[:], AF.Sigmoid, bias=b1_sb[:, e, dc : dc + 1]
                        )
                        nc.vector.scalar_tensor_tensor(
                            hs[:, dc, :], hp[:], b1_sb[:, e, dc : dc + 1], sg[:],
                            ALU.add, ALU.mult,
                        )
                    eop = peo.tile([128, ST], f32, tag="eo")
                    for dc in range(DC):
                        nc.tensor.matmul(
                            eop[:],
                            w2_sb[:, e, dc, :],
                            hs[:, dc, :],
                            start=(dc == 0),
                            stop=(dc == DC - 1),
                        )
                    gp = pmisc.tile([128, ST], f32, tag="misc")
                    nc.tensor.matmul(gp[:], sel_sb[:, e, :], expt[:])
                    gs = gspool.tile([128, ST], f32, tag="gs")
                    nc.scalar.copy(gs[:], gp[:])
                    if e == 0:
                        nc.vector.scalar_tensor_tensor(
                            acc[:], eop[:], b2_sb[:, e : e + 1], gs[:],
                            ALU.add, ALU.mult,
                        )
                    else:
                        tmp = opool.tile([128, ST], f32, tag="outt")
                        nc.vector.scalar_tensor_tensor(
                            tmp[:], eop[:], b2_sb[:, e : e + 1], gs[:],
                            ALU.add, ALU.mult,
                        )
                        nc.vector.tensor_add(acc[:], acc[:], tmp[:])

                outt = opool.tile([128, ST], f32, tag="outt")
                nc.vector.tensor_tensor(outt[:], acc[:], rbp[:], ALU.mult)
                nc.sync.dma_start(outd[:, i * ST : (i + 1) * ST], outt[:])
                if i + 1 < nst:
                    cur_etg = next_etg
                    if n_onehot:
                        cur_masks = next_masks

    if legalize:
        _legalize_waits(nc)
    # populate .instr bytes for extended-ISA instructions (library reload for
    # dma_gather) — raw Bass skips Bacc's codegen pass; walrus errors with
    # "ISA wrong length" on empty instr otherwise
    mybir.codegen_inst_isa_subclasses(nc)
    return nc


def marshal_inputs(
    x, emb0, emb1, W1, b1, W2, b2, Wg, bg, nst=NST, ncores=NCORES, n_gather=2
):
    """Host-side: cast/reshape full inputs into per-core in_maps."""
    n_tok = ncores * nst * ST
    gathered = [t < n_gather for t in range(2)]
    tables = [emb0, emb1]

    def _wrap_idx(col):
        # dma_gather wrapped layout, tiled 8x across partitions (8 Q7 cores)
        w = (
            col[:n_tok].astype(np.int16).reshape(ncores, nst, ST // 16, 16)
            .transpose(0, 1, 3, 2)
        )
        return np.ascontiguousarray(np.tile(w, (1, 1, 8, 1)))

    def _f16_rows(col):
        return np.ascontiguousarray(
            col[:n_tok].astype(np.float16).reshape(ncores, nst, 1, ST)
        )

    shared = {}
    xh = {}
    for t in range(2):
        if gathered[t]:
            xh[f"x{t}i"] = _wrap_idx(x[:, t])
        else:
            xh[f"x{t}"] = _f16_rows(x[:, t])
    if n_gather:
        shared["embg"] = np.ascontiguousarray(
            np.stack([np.asarray(tables[t]) for t in range(2) if gathered[t]]).astype(
                BF16
            )
        )
    if n_gather < 2:
        onehot_tabs = [np.asarray(tables[t]) for t in range(2) if not gathered[t]]
        shared["embs"] = np.ascontiguousarray(
            np.stack(onehot_tabs)
            .reshape(len(onehot_tabs), VC, 128, DC, 128)
            .transpose(2, 0, 1, 3, 4)
            .astype(BF16)
        )
        shared["ivs"] = np.ascontiguousarray(
            (np.arange(VC)[None, :] * 128 + np.arange(128)[:, None]).astype(np.float32)
        )

    shared["w1s"] = np.ascontiguousarray(
        np.asarray(W1).reshape(E, KC, 128, DC, 128).transpose(0, 2, 1, 3, 4).astype(BF16)
    )
    shared["w2s"] = np.ascontiguousarray(
        np.asarray(W2).reshape(E, DC, 128, OUT).transpose(2, 0, 1, 3).astype(BF16)
    )
    shared["wgs"] = np.ascontiguousarray(
        np.asarray(Wg).reshape(KC, 128, E).transpose(1, 0, 2).astype(BF16)
    )
    shared["b1s"] = np.ascontiguousarray(
        np.asarray(b1).reshape(E, DC, 128).transpose(2, 0, 1).astype(np.float32)
    )
    shared["b2s"] = np.ascontiguousarray(np.asarray(b2).T.astype(np.float32))
    shared["bgs"] = np.ascontiguousarray(np.asarray(bg).reshape(E, 1).astype(np.float32))
    shared["sels"] = np.ascontiguousarray(
        np.broadcast_to(np.eye(E, dtype=np.float32)[:, :, None], (E, E, 128)).astype(
            BF16
        )
    )
    return [{**{k: v[c] for k, v in xh.items()}, **shared} for c in range(ncores)]


def kernel(x, emb0, emb1, W1, b1, W2, b2, Wg, bg):
    global LAST_EXEC_NS
    nc = build_program()
    in_maps = marshal_inputs(x, emb0, emb1, W1, b1, W2, b2, Wg, bg)
    trace = os.environ.get("BASSMOE_TRACE", "0") == "1"
    res = run_bass_kernel_spmd(nc, in_maps, list(range(NCORES)), trace=trace)
    LAST_EXEC_NS = res.exec_time_ns
    out = np.empty((B, OUT), dtype=np.float32)
    for c in range(NCORES):
        out[c * BL : (c + 1) * BL, :] = res.results[c]["out"].T
    return out


# revision 60
# speedup vs baseline: 1.2135x; 1.0003x over previous
"""MoE model (embed -> gate -> 4 dense experts -> softmax combine) on 8 TRN2 cores.

Data-parallel: batch (65536 tokens) sharded 8192/core; expert/gating weights
replicated on every core (SBUF-resident, bf16). All on-chip activations are
kept feature-major ("transposed") so that every matmul consumes operands in
their natural layout:

  e_T[f, t]   = embedding lookup, feature-major, via transposing gather DMAs
                issued one supertile ahead on the otherwise-idle GpSimd SWDGE
                path (fallback: one-hot-mask matmul on the PE).
  h_T[d, t]   = silu(W1[e].T-tiles @ e_T + b1)       (PSUM fp32, evac bf16)
  eo_T[o, t]  = W2[e].T-tiles @ h_T + b2             (PSUM fp32)
  logits[e,t] = Wg.T-tiles @ e_T + bg ; softmax via exp / sum (unnormalized
                weights combined first, one reciprocal row scale at the end)
  out_T[o, t] = (sum_e exp_e * eo_e) * recip         (DVE, fp32)

Output per core is [128, 8192] (feature-major); host transposes on unshard.

bf16 inputs with fp32 PSUM accumulation: end-to-end relative error vs the
fp32 reference is ~0.5%.
"""

import os
import numpy as np
import ml_dtypes

import concourse.bass as bass
import concourse.mybir as mybir
import concourse.tile as tile
from concourse.bass_utils import run_bass_kernel_spmd

BF16 = ml_dtypes.bfloat16

B = 65536
V = 512
D = 1024
IN = 2048
E = 4
OUT = 128
NCORES = 8
BL = B // NCORES          # tokens per core
ST = 512                  # tokens per supertile (max PSUM free dim, fp32)
NST = BL // ST            # supertiles per core
KC = IN // 128            # 16 feature chunks
DC = D // 128             # 8 hidden chunks
VC = V // 128             # 4 vocab chunks

LAST_EXEC_NS = None       # set when BASSMOE_TRACE=1


def _legalize_waits(nc, max_waits=1):
    """This walrus build rejects instructions carrying more than ~1 sync-wait
    command ("Too many sync wait commands", CoreV2/V3GenImpl setupSyncWait).
    Hoist all but the last wait of every instruction onto single-wait NoOps
    placed immediately before it in the same engine's stream."""
    for f in nc.m.functions:
        for bb in f.blocks:
            insts = bb.instructions
            if not any(
                inst.sync_info is not None and len(inst.sync_info.on_wait) > max_waits
                for inst in insts
            ):
                continue
            new = []
            for inst in insts:
                si = inst.sync_info
                waits = list(si.on_wait) if si is not None else []
                if len(waits) > max_waits:
                    for w in waits[:-max_waits]:
                        nop = mybir.InstNoOp(
                            name=f"legw-{nc.next_id()}", ins=[], outs=[]
                        )
                        nop.engine = inst.engine
                        nop.sync_info = mybir.SyncInfo(on_wait=[w], on_update=[])
                        new.append(nop)
                    inst.sync_info = mybir.SyncInfo(
                        on_wait=waits[-max_waits:], on_update=list(si.on_update)
                    )
                new.append(inst)
            bb.instructions = new


def build_program(nst=NST, legalize=True, n_gather=2):
    """n_gather: how many of the 2 embedding tables use the gather-DMA path
    (the rest use the one-hot matmul path)."""
    dt = mybir.dt
    f32, bf16, f16 = dt.float32, dt.bfloat16, dt.float16
    AF = mybir.ActivationFunctionType
    ALU = mybir.AluOpType

    gathered = [t < n_gather for t in range(2)]
    n_onehot = 2 - n_gather

    nc = bass.Bass()

    xd = [None, None]
    for t in range(2):
        if gathered[t]:
            # wrapped gather-idx layout: idx j at [j%16, j//16], replicated
            # across the 8 gpsimd cores
            xd[t] = nc.dram_tensor(
                f"x{t}i", [nst, 128, ST // 16], dt.int16, kind="ExternalInput"
            )
        else:
            xd[t] = nc.dram_tensor(
                f"x{t}", [nst, 1, ST], f16, kind="ExternalInput"
            )
    if n_gather:
        embgd = nc.dram_tensor("embg", [n_gather, V, D], bf16, kind="ExternalInput")
    if n_onehot:
        embd = nc.dram_tensor(
            "embs", [128, n_onehot, VC, DC, 128], bf16, kind="ExternalInput"
        )
        ivd = nc.dram_tensor("ivs", [128, VC], f32, kind="ExternalInput")
    w1d = nc.dram_tensor("w1s", [E, 128, KC, DC, 128], bf16, kind="ExternalInput")
    w2d = nc.dram_tensor("w2s", [128, E, DC, OUT], bf16, kind="ExternalInput")
    wgd = nc.dram_tensor("wgs", [128, KC, E], bf16, kind="ExternalInput")
    b1d = nc.dram_tensor("b1s", [128, E, DC], f32, kind="ExternalInput")
    b2d = nc.dram_tensor("b2s", [128, E], f32, kind="ExternalInput")
    bgd = nc.dram_tensor("bgs", [E, 1], f32, kind="ExternalInput")
    seld = nc.dram_tensor("sels", [E, E, 128], bf16, kind="ExternalInput")
    outd = nc.dram_tensor("out", [128, nst * ST], f32, kind="ExternalOutput")

    with tile.TileContext(nc) as tc:
        with (
            tc.tile_pool(name="const", bufs=1) as cpool,
            tc.tile_pool(name="xt", bufs=2) as xpool,
            tc.tile_pool(name="mask", bufs=1) as mpool,
            tc.tile_pool(name="etg", bufs=2) as etgpool,
            tc.tile_pool(name="et", bufs=1) as etpool,
            tc.tile_pool(name="hs", bufs=1) as hpool,
            tc.tile_pool(name="sm", bufs=2) as smpool,
            tc.tile_pool(name="gsc", bufs=1) as gspool,
            tc.tile_pool(name="sgp", bufs=2) as sgpool,
            tc.tile_pool(name="accp", bufs=2) as apool,
            tc.tile_pool(name="outp", bufs=2) as opool,
            tc.tile_pool(name="pmm", bufs=2, space="PSUM") as pmm,
            tc.tile_pool(name="peo", bufs=2, space="PSUM") as peo,
            tc.tile_pool(name="prb", bufs=2, space="PSUM") as prb,
            tc.tile_pool(name="pmisc", bufs=2, space="PSUM") as pmisc,
        ):
            # --- prologue: supertile 0's embedding inputs first ---
            if n_gather:
                from concourse import library_config

                nc.gpsimd.load_library(library_config.mlp)

                def issue_gather(i, t):
                    """table t embedding rows for supertile i -> feature-major
                    e_T chunk tile, via the GpSimd transposing gather DMA."""
                    xi = xpool.tile([128, ST // 16], dt.int16, tag=f"xi{t}")
                    nc.sync.dma_start(xi[:], xd[t][i])
                    etg = etgpool.tile([128, DC, ST], bf16, tag=f"eTg{t}")
                    nc.gpsimd.dma_gather(
                        out_ap=etg[:],
                        in_ap=embgd[t],
                        idxs_ap=xi[:],
                        num_idxs=ST,
                        num_idxs_reg=ST,
                        elem_size=D,
                        transpose=True,
                    )
                    return etg

            if n_onehot:
                iv_sb = cpool.tile([128, VC], f32)
                nc.sync.dma_start(iv_sb[:], ivd[:])
                ones_f16 = cpool.tile([1, 128], f16)
                nc.vector.memset(ones_f16[:], 1.0)
                x0_pre = []
                for t in range(2):
                    if not gathered[t]:
                        xs = xpool.tile([1, ST], f16, tag=f"x{t}")
                        nc.sync.dma_start(xs[:], xd[t][0])
                        x0_pre.append(xs)
                emb_sb = cpool.tile([128, n_onehot, VC, DC, 128], bf16)
                nc.sync.dma_start(emb_sb[:], embd[:])

            cur_etg = [issue_gather(0, t) if gathered[t] else None for t in range(2)]

            # --- resident weights (DMA queue order = when they are needed) ---
            wg_sb = cpool.tile([128, KC, E], bf16)
            nc.sync.dma_start(wg_sb[:], wgd[:])
            b1_sb = cpool.tile([128, E, DC], f32)
            nc.sync.dma_start(b1_sb[:], b1d[:])
            b2_sb = cpool.tile([128, E], f32)
            nc.sync.dma_start(b2_sb[:], b2d[:])
            bg_sb = cpool.tile([E, 1], f32)
            nc.sync.dma_start(bg_sb[:], bgd[:])
            sel_sb = cpool.tile([E, E, 128], bf16)
            nc.sync.dma_start(sel_sb[:], seld[:])
            w1_sbs = []
            for e in range(E):
                t = cpool.tile([128, KC, DC, 128], bf16, tag=f"w1e{e}")
                w1_sbs.append(t)
            nc.sync.dma_start(w1_sbs[0][:], w1d[0])
            w2_sb = cpool.tile([128, E, DC, OUT], bf16)
            nc.sync.dma_start(w2_sb[:], w2d[:])
            for e in range(1, E):
                nc.sync.dma_start(w1_sbs[e][:], w1d[e])

            ones4_bf = cpool.tile([E, 1], bf16)
            nc.vector.memset(ones4_bf[:], 1.0)
            ones128_bf = cpool.tile([1, 128], bf16)
            nc.vector.memset(ones128_bf[:], 1.0)

            def build_masks(i, preloaded=None):
                """x-broadcast (K=1 matmul) + one-hot compares for the
                one-hot-embedded tables of supertile i."""
                ms = {}
                pi = 0
                for t in range(2):
                    if gathered[t]:
                        continue
                    if preloaded is None:
                        xs = xpool.tile([1, ST], f16, tag=f"x{t}")
                        nc.sync.dma_start(xs[:], xd[t][i])
                    else:
                        xs = preloaded[pi]
                        pi += 1
                    p = pmisc.tile([128, ST], f32, tag="misc")
                    nc.tensor.matmul(p[:], ones_f16[:], xs[:])
                    row = []
                    for vc in range(VC):
                        m = mpool.tile([128, ST], bf16, tag=f"m{t}{vc}")
                        nc.vector.tensor_scalar(
                            m[:], p[:], iv_sb[:, vc : vc + 1], None, ALU.is_equal
                        )
                        row.append(m)
                    ms[t] = row
                return ms

            cur_masks = build_masks(0, preloaded=x0_pre) if n_onehot else {}

            for i in range(nst):
                # --- one-hot embedding matmul -> e_T (one-hot tables) ---
                if n_onehot:
                    eT = etpool.tile([128, n_onehot, DC, ST], bf16, tag="eT")
                    oh = 0
                    for t in range(2):
                        if gathered[t]:
                            continue
                        for dc in range(DC):
                            ps = pmm.tile([128, ST], f32, tag="mm")
                            for vc in range(VC):
                                nc.tensor.matmul(
                                    ps[:],
                                    emb_sb[:, oh, vc, dc, :],
                                    cur_masks[t][vc][:],
                                    start=(vc == 0),
                                    stop=(vc == VC - 1),
                                )
                            nc.scalar.copy(eT[:, oh, dc, :], ps[:])
                        oh += 1

                oh_index = {}
                oh = 0
                for t in range(2):
                    if not gathered[t]:
                        oh_index[t] = oh
                        oh += 1

                def eT_chunk(kc):
                    t, dc = kc // DC, kc % DC
                    if gathered[t]:
                        return cur_etg[t][:, dc, :]
                    return eT[:, oh_index[t], dc, :]

                # --- gating: logits -> exp -> sum -> reciprocal bcast ---
                lp = pmisc.tile([E, ST], f32, tag="misc")
                for kc in range(KC):
                    nc.tensor.matmul(
                        lp[:],
                        wg_sb[:, kc, :],
                        eT_chunk(kc),
                        start=(kc == 0),
                        stop=(kc == KC - 1),
                    )
                expt = smpool.tile([E, ST], bf16, tag="expt")
                nc.scalar.activation(expt[:], lp[:], AF.Exp, bias=bg_sb[:])
                sp = pmisc.tile([1, ST], f32, tag="misc")
                nc.tensor.matmul(sp[:], ones4_bf[:], expt[:])
                rec = smpool.tile([1, ST], f32, tag="rec")
                nc.vector.reciprocal(rec[:], sp[:])
                recb = smpool.tile([1, ST], bf16, tag="recb")
                nc.vector.tensor_copy(recb[:], rec[:])
                rbp = prb.tile([128, ST], f32, tag="rb")
                nc.tensor.matmul(rbp[:], ones128_bf[:], recb[:])

                # prefetch next supertile's embeddings: gather DMAs + mask
                # compares overlap with the expert phase below
                next_etg = [None, None]
                if i + 1 < nst:
                    for t in range(2):
                        if gathered[t]:
                            next_etg[t] = issue_gather(i + 1, t)
                    next_masks = build_masks(i + 1) if n_onehot else {}

                # --- experts ---
                acc = apool.tile([128, ST], f32, tag="acc")
                for e in range(E):
                    hs = hpool.tile([128, DC, ST], bf16, tag="hs")
                    for dc in range(DC):
                        hp = pmm.tile([128, ST], f32, tag="mm")
                        for kc in range(KC):
                            nc.tensor.matmul(
                                hp[:],
                                w1_sbs[e][:, kc, dc, :],
                                eT_chunk(kc),
                                start=(kc == 0),
                                stop=(kc == KC - 1),
                            )
                        sg = sgpool.tile([128, ST], f32, tag="sg")
                        nc.scalar.activation(
                            sg[:], hp[:], AF.Sigmoid, bias=b1_sb[:, e, dc : dc + 1]
                        )
                        nc.vector.scalar_tensor_tensor(
                            hs[:, dc, :], hp[:], b1_sb[:, e, dc : dc + 1], sg[:],
                            ALU.add, ALU.mult,
                        )
                    eop = peo.tile([128, ST], f32, tag="eo")
                    for dc in range(DC):
                        nc.tensor.matmul(
                            eop[:],
                            w2_sb[:, e, dc, :],
                            hs[:, dc, :],
                            start=(dc == 0),
                            stop=(dc == DC - 1),
                        )
                    gp = pmisc.tile([128, ST], f32, tag="misc")
                    nc.tensor.matmul(gp[:], sel_sb[:, e, :], expt[:])
                    gs = gspool.tile([128, ST], f32, tag="gs")
                    nc.scalar.copy(gs[:], gp[:])
                    if e == 0:
                        nc.vector.scalar_tensor_tensor(
                            acc[:], eop[:], b2_sb[:, e : e + 1], gs[:],
                            ALU.add, ALU.mult,
                        )
                    else:
                        tmp = opool.tile([128, ST], f32, tag="outt")
                        nc.vector.scalar_tensor_tensor(
                            tmp[:], eop[:], b2_sb[:, e : e + 1], gs[:],
                            ALU.add, ALU.mult,
                        )
                        nc.vector.tensor_add(acc[:], acc[:], tmp[:])

                outt = opool.tile([128, ST], f32, tag="outt")
                nc.vector.tensor_tensor(outt[:], acc[:], rbp[:], ALU.mult)
                nc.sync.dma_start(outd[:, i * ST : (i + 1) * ST], outt[:])
                if i + 1 < nst:
                    cur_etg = next_etg
                    if n_onehot:
                        cur_masks = next_masks

    if legalize:
        _legalize_waits(nc)
    # populate .instr bytes for extended-ISA instructions (library reload for
    # dma_gather) — raw Bass skips Bacc's codegen pass; walrus errors with
    # "ISA wrong length" on empty instr otherwise
    mybir.codegen_inst_isa_subclasses(nc)
    return nc


def marshal_inputs(
    x, emb0, emb1, W1, b1, W2, b2, Wg, bg, nst=NST, ncores=NCORES, n_gather=2
):
    """Host-side: cast/reshape full inputs into per-core in_maps."""
    n_tok = ncores * nst * ST
    gathered = [t < n_gather for t in range(2)]
    tables = [emb0, emb1]

    def _wrap_idx(col):
        # dma_gather wrapped layout, tiled 8x across partitions (8 Q7 cores)
        w = (
            col[:n_tok].astype(np.int16).reshape(ncores, nst, ST // 16, 16)
            .transpose(0, 1, 3, 2)
        )
        return np.ascontiguousarray(np.tile(w, (1, 1, 8, 1)))

    def _f16_rows(col):
        return np.ascontiguousarray(
            col[:n_tok].astype(np.float16).reshape(ncores, nst, 1, ST)
        )

    shared = {}
    xh = {}
    for t in range(2):
        if gathered[t]:
            xh[f"x{t}i"] = _wrap_idx(x[:, t])
        else:
            xh[f"x{t}"] = _f16_rows(x[:, t])
    if n_gather:
        shared["embg"] = np.ascontiguousarray(
            np.stack([np.asarray(tables[t]) for t in range(2) if gathered[t]]).astype(
                BF16
            )
        )
    if n_gather < 2:
        onehot_tabs = [np.asarray(tables[t]) for t in range(2) if not gathered[t]]
        shared["embs"] = np.ascontiguousarray(
            np.stack(onehot_tabs)
            .reshape(len(onehot_tabs), VC, 128, DC, 128)
            .transpose(2, 0, 1, 3, 4)
            .astype(BF16)
        )
        shared["ivs"] = np.ascontiguousarray(
            (np.arange(VC)[None, :] * 128 + np.arange(128)[:, None]).astype(np.float32)
        )

    shared["w1s"] = np.ascontiguousarray(
        np.asarray(W1).reshape(E, KC, 128, DC, 128).transpose(0, 2, 1, 3, 4).astype(BF16)
    )
    shared["w2s"] = np.ascontiguousarray(
        np.asarray(W2).reshape(E, DC, 128, OUT).transpose(2, 0, 1, 3).astype(BF16)
    )
    shared["wgs"] = np.ascontiguousarray(
        np.asarray(Wg).reshape(KC, 128, E).transpose(1, 0, 2).astype(BF16)
    )
    shared["b1s"] = np.ascontiguousarray(
        np.asarray(b1).reshape(E, DC, 128).transpose(2, 0, 1).astype(np.float32)
    )
    shared["b2s"] = np.ascontiguousarray(np.asarray(b2).T.astype(np.float32))
    shared["bgs"] = np.ascontiguousarray(np.asarray(bg).reshape(E, 1).astype(np.float32))
    shared["sels"] = np.ascontiguousarray(
        np.broadcast_to(np.eye(E, dtype=np.float32)[:, :, None], (E, E, 128)).astype(
            BF16
        )
    )
    return [{**{k: v[c] for k, v in xh.items()}, **shared} for c in range(ncores)]


def kernel(x, emb0, emb1, W1, b1, W2, b2, Wg, bg):
    global LAST_EXEC_NS
    nc = build_program()
    in_maps = marshal_inputs(x, emb0, emb1, W1, b1, W2, b2, Wg, bg)
    trace = os.environ.get("BASSMOE_TRACE", "0") == "1"
    res = run_bass_kernel_spmd(nc, in_maps, list(range(NCORES)), trace=trace)
    LAST_EXEC_NS = res.exec_time_ns
    out = np.empty((B, OUT), dtype=np.float32)
    for c in range(NCORES):
        out[c * BL : (c + 1) * BL, :] = res.results[c]["out"].T
    return out


# revision 62
# speedup vs baseline: 1.2139x; 1.0003x over previous
"""MoE model (embed -> gate -> 4 dense experts -> softmax combine) on 8 TRN2 cores.

Data-parallel: batch (65536 tokens) sharded 8192/core; expert/gating weights
replicated on every core (SBUF-resident, bf16). All on-chip activations are
kept feature-major ("transposed") so that every matmul consumes operands in
their natural layout:

  e_T[f, t]   = embedding lookup, feature-major, via transposing gather DMAs
                issued one supertile ahead on the otherwise-idle GpSimd SWDGE
                path (fallback: one-hot-mask matmul on the PE).
  h_T[d, t]   = silu(W1[e].T-tiles @ e_T + b1)       (PSUM fp32, evac bf16)
  eo_T[o, t]  = W2[e].T-tiles @ h_T + b2             (PSUM fp32)
  logits[e,t] = Wg.T-tiles @ e_T + bg ; softmax via exp / sum (unnormalized
                weights combined first, one reciprocal row scale at the end)
  out_T[o, t] = (sum_e exp_e * eo_e) * recip         (DVE, fp32)

Output per core is [128, 8192] (feature-major); host transposes on unshard.

bf16 inputs with fp32 PSUM accumulation: end-to-end relative error vs the
fp32 reference is ~0.5%.
"""

import os
import numpy as np
import ml_dtypes

import concourse.bass as bass
import concourse.mybir as mybir
import concourse.tile as tile
from concourse.bass_utils import run_bass_kernel_spmd

BF16 = ml_dtypes.bfloat16

B = 65536
V = 512
D = 1024
IN = 2048
E = 4
OUT = 128
NCORES = 8
BL = B // NCORES          # tokens per core
ST = 512                  # tokens per supertile (max PSUM free dim, fp32)
NST = BL // ST            # supertiles per core
KC = IN // 128            # 16 feature chunks
DC = D // 128             # 8 hidden chunks
VC = V // 128             # 4 vocab chunks

LAST_EXEC_NS = None       # set when BASSMOE_TRACE=1


def _legalize_waits(nc, max_waits=1):
    """This walrus build rejects instructions carrying more than ~1 sync-wait
    command ("Too many sync wait commands", CoreV2/V3GenImpl setupSyncWait).
    Hoist all but the last wait of every instruction onto single-wait NoOps
    placed immediately before it in the same engine's stream."""
    for f in nc.m.functions:
        for bb in f.blocks:
            insts = bb.instructions
            if not any(
                inst.sync_info is not None and len(inst.sync_info.on_wait) > max_waits
                for inst in insts
            ):
                continue
            new = []
            for inst in insts:
                si = inst.sync_info
                waits = list(si.on_wait) if si is not None else []
                if len(waits) > max_waits:
                    for w in waits[:-max_waits]:
                        nop = mybir.InstNoOp(
                            name=f"legw-{nc.next_id()}", ins=[], outs=[]
                        )
                        nop.engine = inst.engine
                        nop.sync_info = mybir.SyncInfo(on_wait=[w], on_update=[])
                        new.append(nop)
                    inst.sync_info = mybir.SyncInfo(
                        on_wait=waits[-max_waits:], on_update=list(si.on_update)
                    )
                new.append(inst)
            bb.instructions = new


def build_program(nst=NST, legalize=True, n_gather=2):
    """n_gather: how many of the 2 embedding tables use the gather-DMA path
    (the rest use the one-hot matmul path)."""
    dt = mybir.dt
    f32, bf16, f16 = dt.float32, dt.bfloat16, dt.float16
    AF = mybir.ActivationFunctionType
    ALU = mybir.AluOpType

    gathered = [t < n_gather for t in range(2)]
    n_onehot = 2 - n_gather

    nc = bass.Bass()

    xd = [None, None]
    for t in range(2):
        if gathered[t]:
            # wrapped gather-idx layout: idx j at [j%16, j//16], replicated
            # across the 8 gpsimd cores
            xd[t] = nc.dram_tensor(
                f"x{t}i", [nst, 128, ST // 16], dt.int16, kind="ExternalInput"
            )
        else:
            xd[t] = nc.dram_tensor(
                f"x{t}", [nst, 1, ST], f16, kind="ExternalInput"
            )
    if n_gather:
        embgd = nc.dram_tensor("embg", [n_gather, V, D], bf16, kind="ExternalInput")
    if n_onehot:
        embd = nc.dram_tensor(
            "embs", [128, n_onehot, VC, DC, 128], bf16, kind="ExternalInput"
        )
        ivd = nc.dram_tensor("ivs", [128, VC], f32, kind="ExternalInput")
    w1d = nc.dram_tensor("w1s", [E, 128, KC, DC, 128], bf16, kind="ExternalInput")
    w2d = nc.dram_tensor("w2s", [128, E, DC, OUT], bf16, kind="ExternalInput")
    wgd = nc.dram_tensor("wgs", [128, KC, E], bf16, kind="ExternalInput")
    b1d = nc.dram_tensor("b1s", [128, E, DC], f32, kind="ExternalInput")
    b2d = nc.dram_tensor("b2s", [128, E], f32, kind="ExternalInput")
    bgd = nc.dram_tensor("bgs", [E, 1], f32, kind="ExternalInput")
    seld = nc.dram_tensor("sels", [E, E, 128], bf16, kind="ExternalInput")
    outd = nc.dram_tensor("out", [128, nst * ST], f32, kind="ExternalOutput")

    with tile.TileContext(nc) as tc:
        with (
            tc.tile_pool(name="const", bufs=1) as cpool,
            tc.tile_pool(name="xt", bufs=2) as xpool,
            tc.tile_pool(name="mask", bufs=1) as mpool,
            tc.tile_pool(name="etg", bufs=2) as etgpool,
            tc.tile_pool(name="et", bufs=1) as etpool,
            tc.tile_pool(name="hs", bufs=1) as hpool,
            tc.tile_pool(name="sm", bufs=2) as smpool,
            tc.tile_pool(name="gsc", bufs=1) as gspool,
            tc.tile_pool(name="sgp", bufs=2) as sgpool,
            tc.tile_pool(name="accp", bufs=2) as apool,
            tc.tile_pool(name="outp", bufs=2) as opool,
            tc.tile_pool(name="pmm", bufs=2, space="PSUM") as pmm,
            tc.tile_pool(name="peo", bufs=2, space="PSUM") as peo,
            tc.tile_pool(name="prb", bufs=2, space="PSUM") as prb,
            tc.tile_pool(name="pmisc", bufs=2, space="PSUM") as pmisc,
        ):
            # --- prologue: supertile 0's embedding inputs first ---
            if n_gather:
                from concourse import library_config

                nc.gpsimd.load_library(library_config.mlp)

                def issue_gather(i, t):
                    """table t embedding rows for supertile i -> feature-major
                    e_T chunk tile, via the GpSimd transposing gather DMA."""
                    xi = xpool.tile([128, ST // 16], dt.int16, tag=f"xi{t}")
                    nc.sync.dma_start(xi[:], xd[t][i])
                    etg = etgpool.tile([128, DC, ST], bf16, tag=f"eTg{t}")
                    nc.gpsimd.dma_gather(
                        out_ap=etg[:],
                        in_ap=embgd[t],
                        idxs_ap=xi[:],
                        num_idxs=ST,
                        num_idxs_reg=ST,
                        elem_size=D,
                        transpose=True,
                    )
                    return etg

            if n_onehot:
                iv_sb = cpool.tile([128, VC], f32)
                nc.sync.dma_start(iv_sb[:], ivd[:])
                ones_f16 = cpool.tile([1, 128], f16)
                nc.vector.memset(ones_f16[:], 1.0)
                x0_pre = []
                for t in range(2):
                    if not gathered[t]:
                        xs = xpool.tile([1, ST], f16, tag=f"x{t}")
                        nc.sync.dma_start(xs[:], xd[t][0])
                        x0_pre.append(xs)
                emb_sb = cpool.tile([128, n_onehot, VC, DC, 128], bf16)
                nc.sync.dma_start(emb_sb[:], embd[:])

            cur_etg = [issue_gather(0, t) if gathered[t] else None for t in range(2)]

            # --- resident weights (DMA queue order = when they are needed) ---
            wg_sb = cpool.tile([128, KC, E], bf16)
            nc.sync.dma_start(wg_sb[:], wgd[:])
            b1_sb = cpool.tile([128, E, DC], f32)
            nc.sync.dma_start(b1_sb[:], b1d[:])
            b2_sb = cpool.tile([128, E], f32)
            nc.sync.dma_start(b2_sb[:], b2d[:])
            bg_sb = cpool.tile([E, 1], f32)
            nc.sync.dma_start(bg_sb[:], bgd[:])
            sel_sb = cpool.tile([E, E, 128], bf16)
            nc.sync.dma_start(sel_sb[:], seld[:])
            w1_sbs = []
            for e in range(E):
                t = cpool.tile([128, KC, DC, 128], bf16, tag=f"w1e{e}")
                w1_sbs.append(t)
            nc.sync.dma_start(w1_sbs[0][:], w1d[0])
            w2_sb = cpool.tile([128, E, DC, OUT], bf16)
            nc.sync.dma_start(w2_sb[:], w2d[:])
            for e in range(1, E):
                nc.sync.dma_start(w1_sbs[e][:], w1d[e])

            ones4_bf = cpool.tile([E, 1], bf16)
            nc.vector.memset(ones4_bf[:], 1.0)
            ones128_bf = cpool.tile([1, 128], bf16)
            nc.vector.memset(ones128_bf[:], 1.0)

            def build_masks(i, preloaded=None):
                """x-broadcast (K=1 matmul) + one-hot compares for the
                one-hot-embedded tables of supertile i."""
                ms = {}
                pi = 0
                for t in range(2):
                    if gathered[t]:
                        continue
                    if preloaded is None:
                        xs = xpool.tile([1, ST], f16, tag=f"x{t}")
                        nc.sync.dma_start(xs[:], xd[t][i])
                    else:
                        xs = preloaded[pi]
                        pi += 1
                    p = pmisc.tile([128, ST], f32, tag="misc")
                    nc.tensor.matmul(p[:], ones_f16[:], xs[:])
                    row = []
                    for vc in range(VC):
                        m = mpool.tile([128, ST], bf16, tag=f"m{t}{vc}")
                        nc.vector.tensor_scalar(
                            m[:], p[:], iv_sb[:, vc : vc + 1], None, ALU.is_equal
                        )
                        row.append(m)
                    ms[t] = row
                return ms

            cur_masks = build_masks(0, preloaded=x0_pre) if n_onehot else {}

            for i in range(nst):
                # --- one-hot embedding matmul -> e_T (one-hot tables) ---
                if n_onehot:
                    eT = etpool.tile([128, n_onehot, DC, ST], bf16, tag="eT")
                    oh = 0
                    for t in range(2):
                        if gathered[t]:
                            continue
                        for dc in range(DC):
                            ps = pmm.tile([128, ST], f32, tag="mm")
                            for vc in range(VC):
                                nc.tensor.matmul(
                                    ps[:],
                                    emb_sb[:, oh, vc, dc, :],
                                    cur_masks[t][vc][:],
                                    start=(vc == 0),
                                    stop=(vc == VC - 1),
                                )
                            nc.scalar.copy(eT[:, oh, dc, :], ps[:])
                        oh += 1

                oh_index = {}
                oh = 0
                for t in range(2):
                    if not gathered[t]:
                        oh_index[t] = oh
                        oh += 1

                def eT_chunk(kc):
                    t, dc = kc // DC, kc % DC
                    if gathered[t]:
                        return cur_etg[t][:, dc, :]
                    return eT[:, oh_index[t], dc, :]

                # --- gating: logits -> exp -> sum -> reciprocal bcast ---
                lp = pmisc.tile([E, ST], f32, tag="misc")
                for kc in range(KC):
                    nc.tensor.matmul(
                        lp[:],
                        wg_sb[:, kc, :],
                        eT_chunk(kc),
                        start=(kc == 0),
                        stop=(kc == KC - 1),
                    )
                expt = smpool.tile([E, ST], bf16, tag="expt")
                nc.scalar.activation(expt[:], lp[:], AF.Exp, bias=bg_sb[:])

                def emit_recip_chain():
                    # sum-exp -> reciprocal -> bf16 -> broadcast to 128 rows.
                    # Emitted between expert 0 and 1 so the slow single-
                    # partition RECIPROCAL (~3.3us DVE) and the Exp/Sigmoid
                    # ACT-table switch hide under expert-0's W1 matmuls
                    # instead of stalling the PE at the supertile boundary.
                    sp = pmisc.tile([1, ST], f32, tag="misc")
                    nc.tensor.matmul(sp[:], ones4_bf[:], expt[:])
                    rec = smpool.tile([1, ST], f32, tag="rec")
                    nc.vector.reciprocal(rec[:], sp[:])
                    recb = smpool.tile([1, ST], bf16, tag="recb")
                    nc.vector.tensor_copy(recb[:], rec[:])
                    rbp = prb.tile([128, ST], f32, tag="rb")
                    nc.tensor.matmul(rbp[:], ones128_bf[:], recb[:])
                    return rbp

                # prefetch next supertile's embeddings: gather DMAs + mask
                # compares overlap with the expert phase below
                next_etg = [None, None]
                if i + 1 < nst:
                    for t in range(2):
                        if gathered[t]:
                            next_etg[t] = issue_gather(i + 1, t)
                    next_masks = build_masks(i + 1) if n_onehot else {}

                # --- experts ---
                acc = apool.tile([128, ST], f32, tag="acc")
                for e in range(E):
                    if e == 1:
                        rbp = emit_recip_chain()
                    hs = hpool.tile([128, DC, ST], bf16, tag="hs")
                    for dc in range(DC):
                        hp = pmm.tile([128, ST], f32, tag="mm")
                        for kc in range(KC):
                            nc.tensor.matmul(
                                hp[:],
                                w1_sbs[e][:, kc, dc, :],
                                eT_chunk(kc),
                                start=(kc == 0),
                                stop=(kc == KC - 1),
                            )
                        sg = sgpool.tile([128, ST], f32, tag="sg")
                        nc.scalar.activation(
                            sg[:], hp[:], AF.Sigmoid, bias=b1_sb[:, e, dc : dc + 1]
                        )
                        nc.vector.scalar_tensor_tensor(
                            hs[:, dc, :], hp[:], b1_sb[:, e, dc : dc + 1], sg[:],
                            ALU.add, ALU.mult,
                        )
                    eop = peo.tile([128, ST], f32, tag="eo")
                    for dc in range(DC):
                        nc.tensor.matmul(
                            eop[:],
                            w2_sb[:, e, dc, :],
                            hs[:, dc, :],
                            start=(dc == 0),
                            stop=(dc == DC - 1),
                        )
                    gp = pmisc.tile([128, ST], f32, tag="misc")
                    nc.tensor.matmul(gp[:], sel_sb[:, e, :], expt[:])
                    gs = gspool.tile([128, ST], f32, tag="gs")
                    nc.scalar.copy(gs[:], gp[:])
                    if e == 0:
                        nc.vector.scalar_tensor_tensor(
                            acc[:], eop[:], b2_sb[:, e : e + 1], gs[:],
                            ALU.add, ALU.mult,
                        )
                    else:
                        tmp = opool.tile([128, ST], f32, tag="outt")
                        nc.vector.scalar_tensor_tensor(
                            tmp[:], eop[:], b2_sb[:, e : e + 1], gs[:],
                            ALU.add, ALU.mult,
                        )
                        nc.vector.tensor_add(acc[:], acc[:], tmp[:])

                outt = opool.tile([128, ST], f32, tag="outt")
                nc.vector.tensor_tensor(outt[:], acc[:], rbp[:], ALU.mult)
                nc.sync.dma_start(outd[:, i * ST : (i + 1) * ST], outt[:])
                if i + 1 < nst:
                    cur_etg = next_etg
                    if n_onehot:
                        cur_masks = next_masks

    if legalize:
        _legalize_waits(nc)
    # populate .instr bytes for extended-ISA instructions (library reload for
    # dma_gather) — raw Bass skips Bacc's codegen pass; walrus errors with
    # "ISA wrong length" on empty instr otherwise
    mybir.codegen_inst_isa_subclasses(nc)
    return nc


def marshal_inputs(
    x, emb0, emb1, W1, b1, W2, b2, Wg, bg, nst=NST, ncores=NCORES, n_gather=2
):
    """Host-side: cast/reshape full inputs into per-core in_maps."""
    n_tok = ncores * nst * ST
    gathered = [t < n_gather for t in range(2)]
    tables = [emb0, emb1]

    def _wrap_idx(col):
        # dma_gather wrapped layout, tiled 8x across partitions (8 Q7 cores)
        w = (
            col[:n_tok].astype(np.int16).reshape(ncores, nst, ST // 16, 16)
            .transpose(0, 1, 3, 2)
        )
        return np.ascontiguousarray(np.tile(w, (1, 1, 8, 1)))

    def _f16_rows(col):
        return np.ascontiguousarray(
            col[:n_tok].astype(np.float16).reshape(ncores, nst, 1, ST)
        )

    shared = {}
    xh = {}
    for t in range(2):
        if gathered[t]:
            xh[f"x{t}i"] = _wrap_idx(x[:, t])
        else:
            xh[f"x{t}"] = _f16_rows(x[:, t])
    if n_gather:
        shared["embg"] = np.ascontiguousarray(
            np.stack([np.asarray(tables[t]) for t in range(2) if gathered[t]]).astype(
                BF16
            )
        )
    if n_gather < 2:
        onehot_tabs = [np.asarray(tables[t]) for t in range(2) if not gathered[t]]
        shared["embs"] = np.ascontiguousarray(
            np.stack(onehot_tabs)
            .reshape(len(onehot_tabs), VC, 128, DC, 128)
            .transpose(2, 0, 1, 3, 4)
            .astype(BF16)
        )
        shared["ivs"] = np.ascontiguousarray(
            (np.arange(VC)[None, :] * 128 + np.arange(128)[:, None]).astype(np.float32)
        )

    shared["w1s"] = np.ascontiguousarray(
        np.asarray(W1).reshape(E, KC, 128, DC, 128).transpose(0, 2, 1, 3, 4).astype(BF16)
    )
    shared["w2s"] = np.ascontiguousarray(
        np.asarray(W2).reshape(E, DC, 128, OUT).transpose(2, 0, 1, 3).astype(BF16)
    )
    shared["wgs"] = np.ascontiguousarray(
        np.asarray(Wg).reshape(KC, 128, E).transpose(1, 0, 2).astype(BF16)
    )
    shared["b1s"] = np.ascontiguousarray(
        np.asarray(b1).reshape(E, DC, 128).transpose(2, 0, 1).astype(np.float32)
    )
    shared["b2s"] = np.ascontiguousarray(np.asarray(b2).T.astype(np.float32))
    shared["bgs"] = np.ascontiguousarray(np.asarray(bg).reshape(E, 1).astype(np.float32))
    shared["sels"] = np.ascontiguousarray(
        np.broadcast_to(np.eye(E, dtype=np.float32)[:, :, None], (E, E, 128)).astype(
            BF16
        )
    )
    return [{**{k: v[c] for k, v in xh.items()}, **shared} for c in range(ncores)]


def kernel(x, emb0, emb1, W1, b1, W2, b2, Wg, bg):
    global LAST_EXEC_NS
    nc = build_program()
    in_maps = marshal_inputs(x, emb0, emb1, W1, b1, W2, b2, Wg, bg)
    trace = os.environ.get("BASSMOE_TRACE", "0") == "1"
    res = run_bass_kernel_spmd(nc, in_maps, list(range(NCORES)), trace=trace)
    LAST_EXEC_NS = res.exec_time_ns
    out = np.empty((B, OUT), dtype=np.float32)
    for c in range(NCORES):
        out[c * BL : (c + 1) * BL, :] = res.results[c]["out"].T
    return out
